# revision 26
# baseline (speedup 1.0000x reference)
"""Trainium2 Bass kernel for nn_Attention_40767829574409.

Data-parallel over batch N=32 across 8 NeuronCores (4 samples/core).
Device computes (per core, per sample):
  - two 6-layer conv(3,pad1)+BN stacks (BN stats exact via AllGather of
    per-core partial sums across the 8 cores), residual every odd layer,
    relu except last, per-layer length masking
  - query head:  qT = q_w @ y5 + q_b ; qn2[t] = sum_h qT[h,t]^2
  - cosine numerator D[x,t] = keyn[x,:] @ qT[:,t]  (keyn host-normalized)
  - ctc head: ctc_logitT = ctc_w @ y5_aux
Host computes the cheap/sequential remainder: cosine division, similarity,
log-sigmoid losses, monotonic-alignment DP (maximum_path), CTC loss,
silence-promotion loss. Returns (attention, att_loss, att_mask, nll).
"""

import os
import sys

import numpy as np

for _p in ("/opt/trn_rl_repo", "/opt/trn_rl_repo/concourse"):
    if _p not in sys.path:
        sys.path.insert(0, _p)

# problem constants (hardcoded per spec)
NCORES = 8
N = 32
NL = N // NCORES          # samples per core
T = 1200                  # T_DEC
TP = T + 2                # padded time axis (zero col at 0 and T+1)
TX = 240                  # T_TEXT
MEL = 80
HID = 128
C = 256                   # ENC_HID
V = 100                   # VOCAB
TCH = 400                 # matmul free-dim chunk
NT3 = T // TCH            # 3 chunks
CNT = float(N * T)        # BatchNorm element count per channel
EPS = 1e-5
NEG = -1e9

_NC = None                # cached Bass graph


# ---------------------------------------------------------------------------
# device graph
# ---------------------------------------------------------------------------

def _build_nc():
    import concourse.bacc as bacc
    import concourse.tile as tile
    from concourse import mybir
    from contextlib import ExitStack

    f32 = mybir.dt.float32
    ALU = mybir.AluOpType
    ACT = mybir.ActivationFunctionType
    AX = mybir.AxisListType

    nc = bacc.Bacc("TRN2", target_bir_lowering=False, num_devices=NCORES)

    # ---- I/O ----
    spec_pm = nc.declare_dram_parameter("spec_pm", [NL, MEL, TP], f32, False)
    mask_b = nc.declare_dram_parameter("mask_b", [NL, 128, TP], f32, False)
    keynT = nc.declare_dram_parameter("keynT", [NL, HID, TX], f32, False)
    w0_m = nc.declare_dram_parameter("w0_m", [3, MEL, C], f32, False)
    w_m = nc.declare_dram_parameter("w_m", [5, 3, C, C], f32, False)
    w0_a = nc.declare_dram_parameter("w0_a", [3, MEL, C], f32, False)
    w_a = nc.declare_dram_parameter("w_a", [5, 3, C, C], f32, False)
    bng_m = nc.declare_dram_parameter("bng_m", [6, C, 1], f32, False)
    bnb_m = nc.declare_dram_parameter("bnb_m", [6, C, 1], f32, False)
    bng_a = nc.declare_dram_parameter("bng_a", [6, C, 1], f32, False)
    bnb_a = nc.declare_dram_parameter("bnb_a", [6, C, 1], f32, False)
    qwT = nc.declare_dram_parameter("qwT", [C, HID], f32, False)
    qb = nc.declare_dram_parameter("qb", [HID, 1], f32, False)
    ctcwT = nc.declare_dram_parameter("ctcwT", [C, V], f32, False)

    d_out = nc.declare_dram_parameter("d_out", [NL, TX, T], f32, True)
    qn2_out = nc.declare_dram_parameter("qn2_out", [NL, T], f32, True)
    ctc_out = nc.declare_dram_parameter("ctc_out", [NL, V, T], f32, True)

    with tile.TileContext(nc) as tc, ExitStack() as ctx:
        const_pool = ctx.enter_context(tc.tile_pool(name="const", bufs=1))
        mask_pool = ctx.enter_context(tc.tile_pool(name="maskp", bufs=1))
        px_pool = ctx.enter_context(tc.tile_pool(name="px", bufs=2))
        res_pool = ctx.enter_context(tc.tile_pool(name="res", bufs=1))
        w_pool = ctx.enter_context(tc.tile_pool(name="wp", bufs=2))
        sc_pool = ctx.enter_context(tc.tile_pool(name="scr", bufs=2))
        st_pool = ctx.enter_context(tc.tile_pool(name="st", bufs=2))
        qs_pool = ctx.enter_context(tc.tile_pool(name="qs", bufs=2))
        ps_pool = ctx.enter_context(tc.tile_pool(name="ps", bufs=6, space="PSUM"))
        hd_pool = ctx.enter_context(tc.tile_pool(name="hd", bufs=2, space="PSUM"))
        dr_pool = ctx.enter_context(tc.tile_pool(name="dr", bufs=2, space="DRAM"))

        # ---- resident constants ----
        mask_t = []
        for s in range(NL):
            mt = mask_pool.tile([128, TP], f32, name=f"mask{s}", tag=f"mask{s}")
            nc.sync.dma_start(out=mt[:, :], in_=mask_b[s])
            mask_t.append(mt)
        keyn_t = []
        for s in range(NL):
            kt = const_pool.tile([HID, TX], f32, name=f"keyn{s}", tag=f"keyn{s}")
            nc.sync.dma_start(out=kt[:, :], in_=keynT[s])
            keyn_t.append(kt)
        qw_t = []
        for ic in range(2):
            qt = const_pool.tile([128, HID], f32, name=f"qw{ic}", tag=f"qw{ic}")
            nc.sync.dma_start(out=qt[:, :], in_=qwT[ic * 128:(ic + 1) * 128, :])
            qw_t.append(qt)
        qb_t = const_pool.tile([HID, 1], f32, name="qbt", tag="qbt")
        nc.sync.dma_start(out=qb_t[:, :], in_=qb[:, :])
        ctcw_t = []
        for ic in range(2):
            ct = const_pool.tile([128, V], f32, name=f"ctcw{ic}", tag=f"ctcw{ic}")
            nc.sync.dma_start(out=ct[:, :], in_=ctcwT[ic * 128:(ic + 1) * 128, :])
            ctcw_t.append(ct)
        ones_t = const_pool.tile([128, 1], f32, name="ones", tag="ones")
        nc.vector.memset(ones_t[:, :], 1.0)
        eps_t = const_pool.tile([128, 1], f32, name="epst", tag="epst")
        nc.vector.memset(eps_t[:, :], EPS)

        n_layers = int(os.environ.get("KB_LAYERS", "6"))
        n_stacks = int(os.environ.get("KB_STACKS", "2"))
        do_heads = os.environ.get("KB_HEADS", "1") == "1"
        no_stats = os.environ.get("KB_NOSTATS") == "1"
        stats_lvl = int(os.environ.get("KB_STATS_LVL", "4"))
        no_acc = os.environ.get("KB_NOACC") == "1"
        no_apply = os.environ.get("KB_NOAPPLY") == "1"
        no_memset = os.environ.get("KB_NOMEMSET") == "1"

        def conv_stack(sname, w0_h, w_h, g_h, b_h):
            """Runs the 6-layer stack; returns {(s, oc): y5 AP [128, T]}."""
            # layer-0 inputs
            pxin = {}
            for s in range(NL):
                p0 = px_pool.tile([MEL, TP], f32, name=f"{sname}px0_{s}",
                                  tag=f"px_{s}_0")
                nc.sync.dma_start(out=p0[:, :], in_=spec_pm[s])
                pxin[(s, 0)] = p0
            res_t = {}

            for l in range(n_layers):
                nic = 1 if l == 0 else 2
                # weights
                wt = {}
                for k3 in range(3):
                    for ic in range(nic):
                        kdim = MEL if l == 0 else 128
                        w = w_pool.tile([kdim, C], f32,
                                        name=f"{sname}w{l}_{k3}_{ic}",
                                        tag=f"w{k3}{ic}")
                        if l == 0:
                            nc.sync.dma_start(out=w[:, :], in_=w0_h[k3])
                        else:
                            nc.sync.dma_start(
                                out=w[:, :],
                                in_=w_h[l - 1, k3, ic * 128:(ic + 1) * 128, :])
                        wt[(k3, ic)] = w
                # bn params
                gt, bt = [], []
                for oc in range(2):
                    g = st_pool.tile([128, 1], f32, name=f"{sname}g{l}{oc}",
                                     tag=f"g{oc}")
                    nc.sync.dma_start(out=g[:, :],
                                      in_=g_h[l, oc * 128:(oc + 1) * 128, :])
                    b = st_pool.tile([128, 1], f32, name=f"{sname}b{l}{oc}",
                                     tag=f"b{oc}")
                    nc.sync.dma_start(out=b[:, :],
                                      in_=b_h[l, oc * 128:(oc + 1) * 128, :])
                    gt.append(g)
                    bt.append(b)

                # partial stats: cols 0..23 sums per (s,oc,t3); 24..31 sq per (s,oc)
                pt = st_pool.tile([128, 32], f32, name=f"{sname}pt{l}", tag="pt")

                newpx = {}
                for s in range(NL):
                    for oc in range(2):
                        tnew = px_pool.tile([128, TP], f32,
                                            name=f"{sname}px{l + 1}_{s}_{oc}",
                                            tag=f"px_{s}_{oc}")
                        if not no_memset:
                            nc.gpsimd.memset(tnew[:, 0:1], 0.0)
                            nc.gpsimd.memset(tnew[:, TP - 1:TP], 0.0)
                        for t3 in range(NT3):
                            psum = ps_pool.tile([128, TCH], f32,
                                                name=f"{sname}ps{l}{s}{oc}{t3}",
                                                tag="cv")
                            first = True
                            for ic in range(nic):
                                for k3 in range(3):
                                    nc.tensor.matmul(
                                        psum[:, :],
                                        lhsT=wt[(k3, ic)][:, oc * 128:(oc + 1) * 128],
                                        rhs=pxin[(s, ic)][:, t3 * TCH + k3:
                                                          t3 * TCH + k3 + TCH],
                                        start=first,
                                        stop=(ic == nic - 1 and k3 == 2))
                                    first = False
                            # copy PSUM -> px payload, accumulate row-sum
                            col = s * 6 + oc * 3 + t3
                            if no_acc:
                                nc.vector.tensor_copy(
                                    tnew[:, 1 + t3 * TCH:1 + (t3 + 1) * TCH],
                                    psum[:, :])
                            else:
                                nc.vector.tensor_scalar(
                                    out=tnew[:, 1 + t3 * TCH:1 + (t3 + 1) * TCH],
                                    in0=psum[:, :], scalar1=0.0, scalar2=None,
                                    op0=ALU.add, op1=ALU.add,
                                    accum_out=pt[:, col:col + 1])
                        if not no_stats:
                            # sum of squares over the whole payload
                            scr = sc_pool.tile([128, T], f32,
                                               name=f"{sname}sq{l}{s}{oc}",
                                               tag="scr")
                            sqcol = 24 + s * 2 + oc
                            nc.scalar.activation(
                                scr[:, :], tnew[:, 1:1 + T], ACT.Square,
                                accum_out=pt[:, sqcol:sqcol + 1])
                        newpx[(s, oc)] = tnew

                if no_stats:
                    sc_t, bi_t = [], []
                    for oc in range(2):
                        scl = st_pool.tile([128, 1], f32,
                                           name=f"{sname}sc{l}{oc}", tag=f"sc{oc}")
                        nc.vector.memset(scl[:, :], 1.0)
                        bia = st_pool.tile([128, 1], f32,
                                           name=f"{sname}bi{l}{oc}", tag=f"bi{oc}")
                        nc.vector.memset(bia[:, :], 0.0)
                        sc_t.append(scl)
                        bi_t.append(bia)
                else:
                    # finalize local stats -> ccs [128,4] = (s0, s1, sq0, sq1)
                    tmp8 = st_pool.tile([128, 8], f32, name=f"{sname}tmp8{l}",
                                        tag="tmp8")
                    if stats_lvl < 2:
                        sc_t, bi_t = [], []
                        for oc in range(2):
                            scl = st_pool.tile([128, 1], f32,
                                               name=f"{sname}sc{l}{oc}",
                                               tag=f"sc{oc}")
                            nc.vector.memset(scl[:, :], 1.0)
                            bia = st_pool.tile([128, 1], f32,
                                               name=f"{sname}bi{l}{oc}",
                                               tag=f"bi{oc}")
                            nc.vector.memset(bia[:, :], 0.0)
                            sc_t.append(scl)
                            bi_t.append(bia)
                        pxin = newpx
                        continue
                    nc.vector.tensor_reduce(
                        out=tmp8[:, :],
                        in_=pt[:, 0:24].rearrange("p (s o t) -> p (s o) t",
                                                  s=NL, o=2, t=3),
                        axis=AX.X, op=ALU.add)
                    ccs = st_pool.tile([128, 4], f32, name=f"{sname}ccs{l}",
                                       tag="ccs")
                    nc.vector.tensor_reduce(
                        out=ccs[:, 0:2],
                        in_=tmp8[:, :].rearrange("p (s o) -> p o s", s=NL, o=2),
                        axis=AX.X, op=ALU.add)
                    nc.vector.tensor_reduce(
                        out=ccs[:, 2:4],
                        in_=pt[:, 24:32].rearrange("p (s o) -> p o s", s=NL, o=2),
                        axis=AX.X, op=ALU.add)

                    if stats_lvl < 3:
                        sc_t, bi_t = [], []
                        for oc in range(2):
                            scl = st_pool.tile([128, 1], f32,
                                               name=f"{sname}sc{l}{oc}",
                                               tag=f"sc{oc}")
                            nc.vector.memset(scl[:, :], 1.0)
                            bia = st_pool.tile([128, 1], f32,
                                               name=f"{sname}bi{l}{oc}",
                                               tag=f"bi{oc}")
                            nc.vector.memset(bia[:, :], 0.0)
                            sc_t.append(scl)
                            bi_t.append(bia)
                        pxin = newpx
                        continue
                    # cross-core AllGather + local reduce
                    cci = dr_pool.tile([128, 4], f32, name=f"{sname}cci{l}",
                                       tag="cci")
                    cco = dr_pool.tile([128 * NCORES, 4], f32,
                                       name=f"{sname}cco{l}",
                                       tag="cco", addr_space="Shared")
                    nc.sync.dma_start(out=cci[:, :], in_=ccs[:, :])
                    nc.gpsimd.collective_compute(
                        "AllGather", ALU.bypass,
                        replica_groups=[list(range(NCORES))],
                        ins=[cci.opt()], outs=[cco.opt()])
                    allg = st_pool.tile([128, 4 * NCORES], f32,
                                        name=f"{sname}allg{l}", tag="allg")
                    nc.sync.dma_start(
                        out=allg.rearrange("p (r s) -> p r s", r=NCORES, s=4),
                        in_=cco.rearrange("(r p) s -> p r s", r=NCORES, p=128))
                    gst = st_pool.tile([128, 4], f32, name=f"{sname}gst{l}",
                                       tag="gst")
                    nc.vector.tensor_reduce(
                        out=gst[:, :],
                        in_=allg.rearrange("p (r s) -> p s r", r=NCORES, s=4),
                        axis=AX.X, op=ALU.add)

                    if stats_lvl < 4:
                        sc_t, bi_t = [], []
                        for oc in range(2):
                            scl = st_pool.tile([128, 1], f32,
                                               name=f"{sname}sc{l}{oc}",
                                               tag=f"sc{oc}")
                            nc.vector.memset(scl[:, :], 1.0)
                            bia = st_pool.tile([128, 1], f32,
                                               name=f"{sname}bi{l}{oc}",
                                               tag=f"bi{oc}")
                            nc.vector.memset(bia[:, :], 0.0)
                            sc_t.append(scl)
                            bi_t.append(bia)
                        pxin = newpx
                        continue
                    # per-chunk scale/bias
                    sc_t, bi_t = [], []
                    for oc in range(2):
                        mean = st_pool.tile([128, 1], f32,
                                            name=f"{sname}mn{l}{oc}", tag=f"mn{oc}")
                        nc.scalar.mul(mean[:, :], gst[:, oc:oc + 1], 1.0 / CNT)
                        msq = st_pool.tile([128, 1], f32,
                                           name=f"{sname}ms{l}{oc}", tag=f"ms{oc}")
                        nc.scalar.square(msq[:, :], mean[:, :])
                        var = st_pool.tile([128, 1], f32,
                                           name=f"{sname}vr{l}{oc}", tag=f"vr{oc}")
                        nc.vector.scalar_tensor_tensor(
                            out=var[:, :], in0=gst[:, 2 + oc:3 + oc],
                            scalar=1.0 / CNT, in1=msq[:, :],
                            op0=ALU.mult, op1=ALU.subtract)
                        std = st_pool.tile([128, 1], f32,
                                           name=f"{sname}sd{l}{oc}", tag=f"sd{oc}")
                        nc.scalar.activation(std[:, :], var[:, :], ACT.Sqrt,
                                             bias=eps_t[:, :])
                        rstd = st_pool.tile([128, 1], f32,
                                            name=f"{sname}rs{l}{oc}", tag=f"rs{oc}")
                        nc.vector.reciprocal(rstd[:, :], std[:, :])
                        scl = st_pool.tile([128, 1], f32,
                                           name=f"{sname}sc{l}{oc}", tag=f"sc{oc}")
                        nc.vector.tensor_mul(scl[:, :], gt[oc][:, :], rstd[:, :])
                        mtmp = st_pool.tile([128, 1], f32,
                                            name=f"{sname}mt{l}{oc}", tag=f"mt{oc}")
                        nc.vector.tensor_mul(mtmp[:, :], mean[:, :], scl[:, :])
                        bia = st_pool.tile([128, 1], f32,
                                           name=f"{sname}bi{l}{oc}", tag=f"bi{oc}")
                        nc.vector.tensor_sub(bia[:, :], bt[oc][:, :], mtmp[:, :])
                        sc_t.append(scl)
                        bi_t.append(bia)

                # apply BN (+res/relu/mask) in place
                for s in range(0 if no_apply else NL):
                    for oc in range(2):
                        src = newpx[(s, oc)][:, 1:1 + T]
                        mpay = mask_t[s][:, 1:1 + T]
                        if l % 2 == 0:
                            if l < 5:
                                nc.scalar.activation(src, src, ACT.Relu,
                                                     bias=bi_t[oc][:, :],
                                                     scale=sc_t[oc][:, :])
                                nc.vector.tensor_mul(src, src, mpay)
                        elif l == 1:
                            nc.scalar.activation(src, src, ACT.Identity,
                                                 bias=bi_t[oc][:, :],
                                                 scale=sc_t[oc][:, :])
                            rt = res_pool.tile([128, T], f32,
                                               name=f"{sname}res{s}{oc}",
                                               tag=f"res{s}{oc}")
                            nc.vector.tensor_copy(rt[:, :], src)
                            res_t[(s, oc)] = rt
                            nc.vector.scalar_tensor_tensor(
                                out=src, in0=rt[:, :], scalar=0.0, in1=mpay,
                                op0=ALU.max, op1=ALU.mult)
                        else:  # l in (3, 5)
                            nc.scalar.activation(src, src, ACT.Identity,
                                                 bias=bi_t[oc][:, :],
                                                 scale=sc_t[oc][:, :])
                            rt = res_t[(s, oc)]
                            nc.vector.tensor_add(rt[:, :], rt[:, :], src)
                            if l == 3:
                                nc.vector.scalar_tensor_tensor(
                                    out=src, in0=rt[:, :], scalar=0.0, in1=mpay,
                                    op0=ALU.max, op1=ALU.mult)
                pxin = newpx
            return res_t  # y5 == final residual tiles

        # ---------------- main stack + heads ----------------
        y5m = conv_stack("m", w0_m, w_m, bng_m, bnb_m)

        for s in range(NL if (do_heads and n_layers == 6) else 0):
            qs = qs_pool.tile([128, T], f32, name=f"qs{s}", tag="qs")
            for t3 in range(NT3):
                ph = hd_pool.tile([128, TCH], f32, name=f"qh{s}{t3}", tag="hd")
                for ic in range(2):
                    nc.tensor.matmul(ph[:, :], lhsT=qw_t[ic][:, :],
                                     rhs=y5m[(s, ic)][:, t3 * TCH:(t3 + 1) * TCH],
                                     start=(ic == 0), stop=(ic == 1))
                nc.scalar.activation(qs[:, t3 * TCH:(t3 + 1) * TCH], ph[:, :],
                                     ACT.Identity, bias=qb_t[:, :])
            qsq = sc_pool.tile([128, T], f32, name=f"qsq{s}", tag="scr")
            nc.vector.tensor_mul(qsq[:, :], qs[:, :], qs[:, :])
            qn_sb = st_pool.tile([1, T], f32, name=f"qnsb{s}", tag="qnsb")
            for t3 in range(NT3):
                pq = hd_pool.tile([1, TCH], f32, name=f"pq{s}{t3}", tag="hd")
                nc.tensor.matmul(pq[:, :], lhsT=ones_t[:, :],
                                 rhs=qsq[:, t3 * TCH:(t3 + 1) * TCH],
                                 start=True, stop=True)
                nc.scalar.copy(qn_sb[:, t3 * TCH:(t3 + 1) * TCH], pq[:, :])
            nc.sync.dma_start(out=qn2_out[s:s + 1, :], in_=qn_sb[:, :])
            for mx in range(2):
                m0, msz = (0, 128) if mx == 0 else (128, TX - 128)
                for t3 in range(NT3):
                    pd = hd_pool.tile([msz, TCH], f32, name=f"pd{s}{mx}{t3}",
                                      tag="hd")
                    nc.tensor.matmul(pd[:, :], lhsT=keyn_t[s][:, m0:m0 + msz],
                                     rhs=qs[:, t3 * TCH:(t3 + 1) * TCH],
                                     start=True, stop=True)
                    ds = sc_pool.tile([msz, TCH], f32, name=f"dsb{s}{mx}{t3}",
                                      tag="hsb", bufs=3)
                    nc.scalar.copy(ds[:, :], pd[:, :])
                    nc.sync.dma_start(
                        out=d_out[s, m0:m0 + msz, t3 * TCH:(t3 + 1) * TCH],
                        in_=ds[:, :])

        # ---------------- aux stack + ctc head ----------------
        y5a = conv_stack("a", w0_a, w_a, bng_a, bnb_a) if n_stacks == 2 else None

        for s in range(NL if (n_stacks == 2 and do_heads and n_layers == 6) else 0):
            for t3 in range(NT3):
                pc = hd_pool.tile([V, TCH], f32, name=f"pc{s}{t3}", tag="hd")
                for ic in range(2):
                    nc.tensor.matmul(pc[:, :], lhsT=ctcw_t[ic][:, :],
                                     rhs=y5a[(s, ic)][:, t3 * TCH:(t3 + 1) * TCH],
                                     start=(ic == 0), stop=(ic == 1))
                cs = sc_pool.tile([V, TCH], f32, name=f"csb{s}{t3}",
                                  tag="hsb", bufs=3)
                nc.scalar.copy(cs[:, :], pc[:, :])
                nc.sync.dma_start(out=ctc_out[s, :, t3 * TCH:(t3 + 1) * TCH],
                                  in_=cs[:, :])

    nc.compile()
    return nc


# ---------------------------------------------------------------------------
# host side
# ---------------------------------------------------------------------------

def _prep(text, spec, text_lengths, spec_lengths, text_mask, short_token_mask,
          params):
    """Build per-core device inputs + host context."""
    text = np.asarray(text).astype(np.int64)
    spec = np.asarray(spec, np.float32)
    text_lengths = np.asarray(text_lengths).astype(np.int64)
    spec_lengths = np.asarray(spec_lengths).astype(np.int64)
    text_mask = np.asarray(text_mask, np.float32)
    stm = np.asarray(short_token_mask, np.float32)

    p = {k: params[k] for k in params}
    conv_w = [np.asarray(w, np.float32) for w in p['conv_w']]
    conv_wa = [np.asarray(w, np.float32) for w in p['conv_w_aux']]
    bng_m = np.stack([np.asarray(g, np.float32) for g in p['bn_g']])
    bnb_m = np.stack([np.asarray(b, np.float32) for b in p['bn_b']])
    bng_a = np.stack([np.asarray(g, np.float32) for g in p['bn_g_aux']])
    bnb_a = np.stack([np.asarray(b, np.float32) for b in p['bn_b_aux']])
    emb = np.asarray(p['emb'], np.float32)
    q_w = np.asarray(p['q_w'], np.float32)
    q_b = np.asarray(p['q_b'], np.float32)
    ctc_w = np.asarray(p['ctc_w'], np.float32)
    ctc_b = np.asarray(p['ctc_b'], np.float32)
    sim_w = np.asarray(p['sim_w'], np.float32)
    sim_b = np.asarray(p['sim_b'], np.float32)

    smask = (np.arange(T)[None] < spec_lengths[:, None]).astype(np.float32)
    spec_t = spec.transpose(0, 2, 1) * smask[:, None, :]
    spec_pm = np.zeros((N, MEL, TP), np.float32)
    spec_pm[:, :, 1:1 + T] = spec_t
    mask_b = np.zeros((N, 128, TP), np.float32)
    mask_b[:, :, 1:1 + T] = smask[:, None, :]

    key = emb[text] * text_mask[:, :, None]
    keyn = key / np.maximum(
        np.linalg.norm(key, axis=2, keepdims=True).astype(np.float32), 1e-8)
    keynT = np.ascontiguousarray(keyn.transpose(0, 2, 1), np.float32)

    w0_m = np.ascontiguousarray(
        np.stack([conv_w[0][:, :, k].T for k in range(3)]), np.float32)
    w_m = np.ascontiguousarray(
        np.stack([np.stack([conv_w[l][:, :, k].T for k in range(3)])
                  for l in range(1, 6)]), np.float32)
    w0_a = np.ascontiguousarray(
        np.stack([conv_wa[0][:, :, k].T for k in range(3)]), np.float32)
    w_a = np.ascontiguousarray(
        np.stack([np.stack([conv_wa[l][:, :, k].T for k in range(3)])
                  for l in range(1, 6)]), np.float32)

    shared = dict(
        w0_m=w0_m, w_m=w_m, w0_a=w0_a, w_a=w_a,
        bng_m=np.ascontiguousarray(bng_m[:, :, None], np.float32),
        bnb_m=np.ascontiguousarray(bnb_m[:, :, None], np.float32),
        bng_a=np.ascontiguousarray(bng_a[:, :, None], np.float32),
        bnb_a=np.ascontiguousarray(bnb_a[:, :, None], np.float32),
        qwT=np.ascontiguousarray(q_w.T, np.float32),
        qb=np.ascontiguousarray(q_b[:, None], np.float32),
        ctcwT=np.ascontiguousarray(ctc_w.T, np.float32),
    )
    in_maps = []
    for i in range(NCORES):
        sl = slice(i * NL, (i + 1) * NL)
        m = dict(shared)
        m['spec_pm'] = np.ascontiguousarray(spec_pm[sl])
        m['mask_b'] = np.ascontiguousarray(mask_b[sl])
        m['keynT'] = np.ascontiguousarray(keynT[sl])
        in_maps.append(m)

    host = dict(text=text, spec=spec, text_lengths=text_lengths,
                spec_lengths=spec_lengths, text_mask=text_mask, stm=stm,
                ctc_b=ctc_b, sim_w=sim_w, sim_b=sim_b)
    return in_maps, host


def _log_sigmoid(x):
    return -np.logaddexp(np.float32(0.0), -x)


def _maximum_path(value, t_x, t_y):
    """numpy port of the Glow-TTS monotonic alignment search (f32)."""
    Nb, Tx, Ty = value.shape
    xs = np.arange(Tx)[None]
    dp = np.full((Nb, Tx), NEG, np.float32)
    dp_table = np.empty((Nb, Tx, Ty), np.float32)
    tx = t_x[:, None]
    ty = t_y[:, None]
    for y in range(Ty):
        v = value[:, :, y]
        shifted = np.concatenate(
            [np.full((Nb, 1), NEG, np.float32), dp[:, :-1]], axis=1)
        best = np.maximum(dp, shifted)
        if y == 0:
            base = np.where(xs == 0, np.float32(0.0), np.float32(NEG))
        else:
            base = best
        dp_new = v + base
        valid = (xs <= y) & (xs >= tx + y - ty) & (xs < tx) & (y < ty)
        dp = np.where(valid, dp_new, np.float32(NEG)).astype(np.float32)
        dp_table[:, :, y] = dp
    bi = np.arange(Nb)
    idx = (t_x - 1).astype(np.int64)
    path = np.zeros((Nb, Tx, Ty), np.float32)
    for y in range(Ty - 1, -1, -1):
        active = y < t_y
        col_prev = dp_table[:, :, max(y - 1, 0)]
        v_cur = col_prev[bi, idx]
        v_prev = col_prev[bi, np.maximum(idx - 1, 0)]
        path[bi, idx, y] = active.astype(np.float32)
        move = active & (idx > 0) & (y > 0) & ((idx == y) | (v_cur < v_prev))
        idx = idx - move.astype(np.int64)
    return path


def _ctc_loss_mean(log_probs, targets, in_lens, tgt_lens):
    """numpy port of the reference CTC loss (f32)."""
    Tt, Nb, Cc = log_probs.shape
    S = targets.shape[1]
    L = 2 * S + 1
    z = np.zeros((Nb, L), targets.dtype)
    z[:, 1::2] = targets
    sidx = np.arange(L)[None]
    z_m2 = np.pad(z, ((0, 0), (2, 0)))[:, :L]
    skip = (sidx % 2 == 1) & (sidx >= 2) & (z != z_m2)

    e0 = np.take_along_axis(log_probs[0], z, axis=1)
    alpha = np.full((Nb, L), NEG, np.float32)
    alpha[:, 0] = e0[:, 0]
    alpha[:, 1] = e0[:, 1]
    negpad = np.full((Nb, 1), NEG, np.float32)
    negpad2 = np.full((Nb, 2), NEG, np.float32)
    for t in range(1, Tt):
        lp = log_probs[t]
        a2 = np.concatenate([negpad, alpha[:, :-1]], axis=1)
        a3 = np.where(skip, np.concatenate([negpad2, alpha[:, :-2]], axis=1),
                      np.float32(NEG))
        new = np.take_along_axis(lp, z, axis=1) + np.logaddexp(
            np.logaddexp(alpha, a2), a3)
        alpha = np.where((t < in_lens)[:, None], new, alpha).astype(np.float32)
    bi = np.arange(Nb)
    ll = np.logaddexp(alpha[bi, 2 * tgt_lens], alpha[bi, 2 * tgt_lens - 1])
    return np.mean(-ll / tgt_lens.astype(ll.dtype))


def _finish(D, qn2, ctcT, host):
    """Everything after the device part. D [N,TX,T]; qn2 [N,T]; ctcT [N,V,T]."""
    text = host['text']
    spec = host['spec']
    text_lengths = host['text_lengths']
    spec_lengths = host['spec_lengths']
    stm = host['stm']

    qnorm = np.sqrt(np.maximum(qn2, 0.0)).astype(np.float32)
    cos = D / np.maximum(qnorm, 1e-8)[:, None, :]
    cos = (1.0 - stm) * cos - stm
    similarity = (10.0 * np.exp(host['sim_w']) * cos + host['sim_b']).astype(
        np.float32)

    ctc_logit = ctcT.transpose(0, 2, 1) + host['ctc_b']  # [N, T, V]
    # softmax over V
    mx = ctc_logit.max(axis=2, keepdims=True)
    ex = np.exp(ctc_logit - mx)
    ctc_query = (ex / ex.sum(axis=2, keepdims=True)).astype(np.float32)
    sim_ctc = np.take_along_axis(
        ctc_query.transpose(0, 2, 1), text[:, :, None], axis=1).astype(np.float32)

    tm_b = np.arange(TX)[None] < text_lengths[:, None]
    sm_b = np.arange(T)[None] < spec_lengths[:, None]
    att_mask = (tm_b[:, :, None] & sm_b[:, None, :]).astype(np.float32)

    logsig = _log_sigmoid(similarity).astype(np.float32)
    lsmx = logsig * att_mask
    lsmx_att = lsmx - (lsmx == 0).astype(np.float32) * lsmx.min()
    match_mask = _maximum_path(lsmx_att, text_lengths, spec_lengths)
    attention = match_mask

    lsmx2 = (_log_sigmoid(sim_ctc) * att_mask).astype(np.float32)
    lsmx_aux = lsmx2 - (lsmx2 == 0).astype(np.float32) * lsmx2.min()
    att_aux = _maximum_path(lsmx_aux, text_lengths, spec_lengths)
    aa = np.pad(att_aux, ((0, 0), (1, 1), (0, 0)))
    att_aux = ((aa[:, :-2] + aa[:, 1:-1] + aa[:, 2:]) * att_mask).astype(
        np.float32)

    neg_logsig = _log_sigmoid(-similarity).astype(np.float32)
    denom = att_mask.sum(axis=(1, 2))
    inter = -(match_mask * logsig + (1 - match_mask) * att_mask * neg_logsig)
    nll = np.float32(np.mean(inter.sum(axis=(1, 2)) / denom))
    att_loss = nll
    aux_l = -(att_aux * logsig + (1 - att_aux) * att_mask * neg_logsig)
    att_loss = att_loss + np.float32(
        np.mean(aux_l.sum(axis=(1, 2)) / denom * 0.5))

    # CTC
    lmx = ctc_logit.max(axis=2, keepdims=True)
    lse = lmx + np.log(np.exp(ctc_logit - lmx).sum(axis=2, keepdims=True))
    ctc_in = np.ascontiguousarray(
        (ctc_logit - lse).transpose(1, 0, 2), np.float32)  # [T, N, V]
    att_loss = att_loss + np.float32(
        _ctc_loss_mean(ctc_in, text, spec_lengths, text_lengths))

    # silence promotion
    tm_i = tm_b.astype(np.int32)
    sm_sil = tm_i.copy()
    sm_sil[:, :-1] += -tm_i[:, 1:]
    sm_sil[:, 0] = 1
    silence = sm_sil[:, :, None].astype(np.float32)
    energy = np.mean(np.exp(spec[:, :, 20:]), axis=2)[:, None, :].astype(
        np.float32)
    se_max = np.max(np.sum(energy * silence * attention, axis=1), axis=1)
    ns_min = np.sum(energy * (1 - silence) * attention, axis=1)
    ns_min = np.min((ns_min == 0).astype(np.float32) * 100 + ns_min, axis=1)
    db = ((se_max + ns_min) / 2)[:, None, None]
    promo = (energy <= db).astype(np.float32) * silence
    sp = -0.01 * np.sum(promo * att_mask * logsig, axis=(1, 2)) / np.maximum(
        np.sum(promo * att_mask, axis=(1, 2)), 1.0)
    att_loss = att_loss + np.float32(np.mean(sp))

    return (attention.astype(np.float32), np.float32(att_loss),
            att_mask, np.float32(nll))


# ---------------------------------------------------------------------------
# numpy twin of the device math (for validation without hardware)
# ---------------------------------------------------------------------------

def _device_twin(in_maps):
    """Replicates the device computation in numpy at full-batch level."""
    spec_pm = np.concatenate([m['spec_pm'] for m in in_maps])   # [N, MEL, TP]
    mask_b = np.concatenate([m['mask_b'] for m in in_maps])
    keynT = np.concatenate([m['keynT'] for m in in_maps])
    sh = in_maps[0]

    def stack(w0, w, g, b):
        px = spec_pm.copy()  # [N, K, TP]
        res = None
        for l in range(6):
            wk = [w0[k] for k in range(3)] if l == 0 else \
                 [w[l - 1, k] for k in range(3)]
            conv = np.zeros((N, C, T), np.float32)
            for k in range(3):
                # out[:, oc, t] += wk[k].T @ px[:, ic, t+k]
                conv += np.matmul(wk[k].T[None], px[:, :, k:k + T])
            s1 = conv.sum(axis=(0, 2))
            s2 = (conv * conv).sum(axis=(0, 2))
            mean = s1 / CNT
            var = s2 / CNT - mean * mean
            scl = g[l, :, 0] / np.sqrt(var + EPS)
            bia = b[l, :, 0] - mean * scl
            y = conv * scl[None, :, None] + bia[None, :, None]
            if l % 2 == 1:
                res = y if res is None else (y + res)
                y = res
            if l < 5:
                nxt = np.zeros((N, C, TP), np.float32)
                nxt[:, :, 1:1 + T] = np.maximum(y, 0.0) * mask_b[:, 0:1, 1:1 + T]
                px = nxt
            else:
                return y
        return None

    y5m = stack(sh['w0_m'], sh['w_m'], sh['bng_m'], sh['bnb_m'])
    y5a = stack(sh['w0_a'], sh['w_a'], sh['bng_a'], sh['bnb_a'])

    qT = np.matmul(sh['qwT'].T[None], y5m) + sh['qb'][None, :, :]
    qn2 = (qT * qT).sum(axis=1)
    D = np.matmul(keynT.transpose(0, 2, 1), qT)
    ctcT = np.matmul(sh['ctcwT'].T[None], y5a)
    return D, qn2, ctcT


# ---------------------------------------------------------------------------
# entry point
# ---------------------------------------------------------------------------

def kernel(**inputs):
    global _NC
    in_maps, host = _prep(**inputs)

    if os.environ.get("KERNEL_TWIN"):
        D, qn2, ctcT = _device_twin(in_maps)
        return _finish(D, qn2, ctcT, host)

    from concourse.bass_utils import run_bass_kernel_spmd
    if _NC is None:
        _NC = _build_nc()
    res = run_bass_kernel_spmd(_NC, in_maps, core_ids=list(range(NCORES)))
    outs = res.results
    D = np.concatenate([np.asarray(o['d_out']) for o in outs])
    qn2 = np.concatenate([np.asarray(o['qn2_out']) for o in outs])
    ctcT = np.concatenate([np.asarray(o['ctc_out']) for o in outs])
    return _finish(D, qn2, ctcT, host)


# revision 30
# speedup vs baseline: 1.2922x; 1.2922x over previous
"""Trainium2 Bass kernel for nn_Attention_40767829574409.

Data-parallel over batch N=32 across 8 NeuronCores (4 samples/core).
Device computes (per core, per sample):
  - two 6-layer conv(3,pad1)+BN stacks (BN stats exact via AllGather of
    per-core partial sums across the 8 cores), residual every odd layer,
    relu except last, per-layer length masking
  - query head:  qT = q_w @ y5 + q_b ; qn2[t] = sum_h qT[h,t]^2
  - cosine numerator D[x,t] = keyn[x,:] @ qT[:,t]  (keyn host-normalized)
  - ctc head: ctc_logitT = ctc_w @ y5_aux
Host computes the cheap/sequential remainder: cosine division, similarity,
log-sigmoid losses, monotonic-alignment DP (maximum_path), CTC loss,
silence-promotion loss. Returns (attention, att_loss, att_mask, nll).
"""

import os
import sys

import numpy as np

for _p in ("/opt/trn_rl_repo", "/opt/trn_rl_repo/concourse"):
    if _p not in sys.path:
        sys.path.insert(0, _p)

# problem constants (hardcoded per spec)
NCORES = 8
N = 32
NL = N // NCORES          # samples per core
T = 1200                  # T_DEC
TP = T + 2                # padded time axis (zero col at 0 and T+1)
TX = 240                  # T_TEXT
MEL = 80
HID = 128
C = 256                   # ENC_HID
V = 100                   # VOCAB
TCH = 400                 # matmul free-dim chunk
NT3 = T // TCH            # 3 chunks
CNT = float(N * T)        # BatchNorm element count per channel
EPS = 1e-5
NEG = -1e9

_NC = None                # cached Bass graph


# ---------------------------------------------------------------------------
# device graph
# ---------------------------------------------------------------------------

def _build_nc():
    import concourse.bacc as bacc
    import concourse.tile as tile
    from concourse import mybir
    from contextlib import ExitStack

    f32 = mybir.dt.float32
    ALU = mybir.AluOpType
    ACT = mybir.ActivationFunctionType
    AX = mybir.AxisListType

    nc = bacc.Bacc("TRN2", target_bir_lowering=False, num_devices=NCORES)

    # ---- I/O ----
    spec_pm = nc.declare_dram_parameter("spec_pm", [NL, MEL, TP], f32, False)
    mask_b = nc.declare_dram_parameter("mask_b", [NL, 128, TP], f32, False)
    keynT = nc.declare_dram_parameter("keynT", [NL, HID, TX], f32, False)
    w0_m = nc.declare_dram_parameter("w0_m", [3, MEL, C], f32, False)
    w_m = nc.declare_dram_parameter("w_m", [5, 3, C, C], f32, False)
    w0_a = nc.declare_dram_parameter("w0_a", [3, MEL, C], f32, False)
    w_a = nc.declare_dram_parameter("w_a", [5, 3, C, C], f32, False)
    bng_m = nc.declare_dram_parameter("bng_m", [6, C, 1], f32, False)
    bnb_m = nc.declare_dram_parameter("bnb_m", [6, C, 1], f32, False)
    bng_a = nc.declare_dram_parameter("bng_a", [6, C, 1], f32, False)
    bnb_a = nc.declare_dram_parameter("bnb_a", [6, C, 1], f32, False)
    qwT = nc.declare_dram_parameter("qwT", [C, HID], f32, False)
    qb = nc.declare_dram_parameter("qb", [HID, 1], f32, False)
    ctcwT = nc.declare_dram_parameter("ctcwT", [C, V], f32, False)

    d_out = nc.declare_dram_parameter("d_out", [NL, TX, T], f32, True)
    qn2_out = nc.declare_dram_parameter("qn2_out", [NL, T], f32, True)
    ctc_out = nc.declare_dram_parameter("ctc_out", [NL, V, T], f32, True)

    with tile.TileContext(nc) as tc, ExitStack() as ctx:
        const_pool = ctx.enter_context(tc.tile_pool(name="const", bufs=1))
        mask_pool = ctx.enter_context(tc.tile_pool(name="maskp", bufs=1))
        px_pool = ctx.enter_context(tc.tile_pool(name="px", bufs=2))
        res_pool = ctx.enter_context(tc.tile_pool(name="res", bufs=1))
        w_pool = ctx.enter_context(tc.tile_pool(name="wp", bufs=2))
        sc_pool = ctx.enter_context(tc.tile_pool(name="scr", bufs=2))
        st_pool = ctx.enter_context(tc.tile_pool(name="st", bufs=2))
        qs_pool = ctx.enter_context(tc.tile_pool(name="qs", bufs=2))
        ps_pool = ctx.enter_context(tc.tile_pool(name="ps", bufs=6, space="PSUM"))
        hd_pool = ctx.enter_context(tc.tile_pool(name="hd", bufs=2, space="PSUM"))
        dr_pool = ctx.enter_context(tc.tile_pool(name="dr", bufs=2, space="DRAM"))

        # ---- resident constants ----
        mask_t = []
        for s in range(NL):
            mt = mask_pool.tile([128, TP], f32, name=f"mask{s}", tag=f"mask{s}")
            nc.sync.dma_start(out=mt[:, :], in_=mask_b[s])
            mask_t.append(mt)
        keyn_t = []
        for s in range(NL):
            kt = const_pool.tile([HID, TX], f32, name=f"keyn{s}", tag=f"keyn{s}")
            nc.sync.dma_start(out=kt[:, :], in_=keynT[s])
            keyn_t.append(kt)
        qw_t = []
        for ic in range(2):
            qt = const_pool.tile([128, HID], f32, name=f"qw{ic}", tag=f"qw{ic}")
            nc.sync.dma_start(out=qt[:, :], in_=qwT[ic * 128:(ic + 1) * 128, :])
            qw_t.append(qt)
        qb_t = const_pool.tile([HID, 1], f32, name="qbt", tag="qbt")
        nc.sync.dma_start(out=qb_t[:, :], in_=qb[:, :])
        ctcw_t = []
        for ic in range(2):
            ct = const_pool.tile([128, V], f32, name=f"ctcw{ic}", tag=f"ctcw{ic}")
            nc.sync.dma_start(out=ct[:, :], in_=ctcwT[ic * 128:(ic + 1) * 128, :])
            ctcw_t.append(ct)
        ones_t = const_pool.tile([128, 1], f32, name="ones", tag="ones")
        nc.vector.memset(ones_t[:, :], 1.0)
        eps_t = const_pool.tile([128, 1], f32, name="epst", tag="epst")
        nc.vector.memset(eps_t[:, :], EPS)

        n_layers = int(os.environ.get("KB_LAYERS", "6"))
        n_stacks = int(os.environ.get("KB_STACKS", "2"))
        do_heads = os.environ.get("KB_HEADS", "1") == "1"
        no_stats = os.environ.get("KB_NOSTATS") == "1"
        stats_lvl = int(os.environ.get("KB_STATS_LVL", "4"))
        no_acc = os.environ.get("KB_NOACC") == "1"
        no_apply = os.environ.get("KB_NOAPPLY") == "1"
        no_memset = os.environ.get("KB_NOMEMSET") == "1"

        dtmap = {"f32": f32, "f32r": mybir.dt.float32r,
                 "bf16": mybir.dt.bfloat16}
        cdt_m = dtmap[os.environ.get("KB_MMDT_MAIN", "f32")]
        cdt_a = dtmap[os.environ.get("KB_MMDT_AUX", "f32")]

        # masks converted per conv dtype (DMA cannot round to f32r/bf16)
        mask_cv = {f32: mask_t}
        for dt_ in {cdt_m, cdt_a} - {f32}:
            cvl = []
            for s in range(NL):
                mc = mask_pool.tile([128, TP], dt_,
                                    name=f"maskc{dt_.value}{s}",
                                    tag=f"maskc{dt_.value}{s}")
                nc.vector.tensor_copy(mc[:, :], mask_t[s][:, :])
                cvl.append(mc)
            mask_cv[dt_] = cvl

        # head weights converted to the owning stack's dtype
        def conv_consts(tiles, dt_, prefix):
            if dt_ == f32:
                return tiles
            out = []
            for i, tl in enumerate(tiles):
                cc = const_pool.tile(list(tl.shape), dt_,
                                     name=f"{prefix}{i}c", tag=f"{prefix}{i}c")
                nc.vector.tensor_copy(cc[:, :], tl[:, :])
                out.append(cc)
            return out

        qw_t = conv_consts(qw_t, cdt_m, "qwc")
        keyn_t = conv_consts(keyn_t, cdt_m, "knc")
        ctcw_t = conv_consts(ctcw_t, cdt_a, "ctc")
        ones_t = conv_consts([ones_t], cdt_m, "one")[0]

        def conv_stack(sname, w0_h, w_h, g_h, b_h, cdt):
            """Runs the 6-layer stack; returns {(s, oc): y5 AP [128, T]}."""
            mask_c = mask_cv[cdt]
            # layer-0 inputs
            pxin = {}
            for s in range(NL):
                p0 = px_pool.tile([MEL, TP], cdt, name=f"{sname}px0_{s}",
                                  tag=f"px_{s}_0")
                if cdt == f32:
                    nc.sync.dma_start(out=p0[:, :], in_=spec_pm[s])
                else:
                    p0f = sc_pool.tile([MEL, TP], f32, name=f"{sname}px0f{s}",
                                       tag="scr")
                    nc.sync.dma_start(out=p0f[:, :], in_=spec_pm[s])
                    nc.vector.tensor_copy(p0[:, :], p0f[:, :])
                pxin[(s, 0)] = p0
            res_t = {}

            for l in range(n_layers):
                nic = 1 if l == 0 else 2
                # weights
                wt = {}
                for k3 in range(3):
                    for ic in range(nic):
                        kdim = MEL if l == 0 else 128
                        src = w0_h[k3] if l == 0 else \
                            w_h[l - 1, k3, ic * 128:(ic + 1) * 128, :]
                        if cdt == f32:
                            w = w_pool.tile([kdim, C], f32,
                                            name=f"{sname}w{l}_{k3}_{ic}",
                                            tag=f"w{k3}{ic}")
                            nc.sync.dma_start(out=w[:, :], in_=src)
                        else:
                            wf = w_pool.tile([kdim, C], f32,
                                             name=f"{sname}wf{l}_{k3}_{ic}",
                                             tag=f"wf{k3}{ic}")
                            nc.sync.dma_start(out=wf[:, :], in_=src)
                            w = w_pool.tile([kdim, C], cdt,
                                            name=f"{sname}w{l}_{k3}_{ic}",
                                            tag=f"w{k3}{ic}")
                            nc.vector.tensor_copy(w[:, :], wf[:, :])
                        wt[(k3, ic)] = w
                # bn params
                gt, bt = [], []
                for oc in range(2):
                    g = st_pool.tile([128, 1], f32, name=f"{sname}g{l}{oc}",
                                     tag=f"g{oc}")
                    nc.sync.dma_start(out=g[:, :],
                                      in_=g_h[l, oc * 128:(oc + 1) * 128, :])
                    b = st_pool.tile([128, 1], f32, name=f"{sname}b{l}{oc}",
                                     tag=f"b{oc}")
                    nc.sync.dma_start(out=b[:, :],
                                      in_=b_h[l, oc * 128:(oc + 1) * 128, :])
                    gt.append(g)
                    bt.append(b)

                # partial stats: cols 0..23 sums per (s,oc,t3); 24..31 sq per (s,oc)
                pt = st_pool.tile([128, 32], f32, name=f"{sname}pt{l}", tag="pt")

                newpx = {}
                for s in range(NL):
                    for oc in range(2):
                        tnew = px_pool.tile([128, TP], cdt,
                                            name=f"{sname}px{l + 1}_{s}_{oc}",
                                            tag=f"px_{s}_{oc}")
                        if not no_memset:
                            nc.gpsimd.memset(tnew[:, 0:1], 0.0)
                            nc.gpsimd.memset(tnew[:, TP - 1:TP], 0.0)
                        for t3 in range(NT3):
                            psum = ps_pool.tile([128, TCH], f32,
                                                name=f"{sname}ps{l}{s}{oc}{t3}",
                                                tag="cv")
                            first = True
                            for ic in range(nic):
                                for k3 in range(3):
                                    nc.tensor.matmul(
                                        psum[:, :],
                                        lhsT=wt[(k3, ic)][:, oc * 128:(oc + 1) * 128],
                                        rhs=pxin[(s, ic)][:, t3 * TCH + k3:
                                                          t3 * TCH + k3 + TCH],
                                        start=first,
                                        stop=(ic == nic - 1 and k3 == 2))
                                    first = False
                            # copy PSUM -> px payload, accumulate row-sum
                            col = s * 6 + oc * 3 + t3
                            if no_acc:
                                nc.vector.tensor_copy(
                                    tnew[:, 1 + t3 * TCH:1 + (t3 + 1) * TCH],
                                    psum[:, :])
                            else:
                                nc.vector.tensor_scalar(
                                    out=tnew[:, 1 + t3 * TCH:1 + (t3 + 1) * TCH],
                                    in0=psum[:, :], scalar1=0.0, scalar2=None,
                                    op0=ALU.add, op1=ALU.add,
                                    accum_out=pt[:, col:col + 1])
                        if not no_stats:
                            # sum of squares over the whole payload
                            scr = sc_pool.tile([128, T], f32,
                                               name=f"{sname}sq{l}{s}{oc}",
                                               tag="scr")
                            sqcol = 24 + s * 2 + oc
                            nc.scalar.activation(
                                scr[:, :], tnew[:, 1:1 + T], ACT.Square,
                                accum_out=pt[:, sqcol:sqcol + 1])
                        newpx[(s, oc)] = tnew

                if no_stats:
                    sc_t, bi_t = [], []
                    for oc in range(2):
                        scl = st_pool.tile([128, 1], f32,
                                           name=f"{sname}sc{l}{oc}", tag=f"sc{oc}")
                        nc.vector.memset(scl[:, :], 1.0)
                        bia = st_pool.tile([128, 1], f32,
                                           name=f"{sname}bi{l}{oc}", tag=f"bi{oc}")
                        nc.vector.memset(bia[:, :], 0.0)
                        sc_t.append(scl)
                        bi_t.append(bia)
                else:
                    # finalize local stats -> ccs [128,4] = (s0, s1, sq0, sq1)
                    tmp8 = st_pool.tile([128, 8], f32, name=f"{sname}tmp8{l}",
                                        tag="tmp8")
                    if stats_lvl < 2:
                        sc_t, bi_t = [], []
                        for oc in range(2):
                            scl = st_pool.tile([128, 1], f32,
                                               name=f"{sname}sc{l}{oc}",
                                               tag=f"sc{oc}")
                            nc.vector.memset(scl[:, :], 1.0)
                            bia = st_pool.tile([128, 1], f32,
                                               name=f"{sname}bi{l}{oc}",
                                               tag=f"bi{oc}")
                            nc.vector.memset(bia[:, :], 0.0)
                            sc_t.append(scl)
                            bi_t.append(bia)
                        pxin = newpx
                        continue
                    nc.vector.tensor_reduce(
                        out=tmp8[:, :],
                        in_=pt[:, 0:24].rearrange("p (s o t) -> p (s o) t",
                                                  s=NL, o=2, t=3),
                        axis=AX.X, op=ALU.add)
                    ccs = st_pool.tile([128, 4], f32, name=f"{sname}ccs{l}",
                                       tag="ccs")
                    nc.vector.tensor_reduce(
                        out=ccs[:, 0:2],
                        in_=tmp8[:, :].rearrange("p (s o) -> p o s", s=NL, o=2),
                        axis=AX.X, op=ALU.add)
                    nc.vector.tensor_reduce(
                        out=ccs[:, 2:4],
                        in_=pt[:, 24:32].rearrange("p (s o) -> p o s", s=NL, o=2),
                        axis=AX.X, op=ALU.add)

                    if stats_lvl < 3:
                        sc_t, bi_t = [], []
                        for oc in range(2):
                            scl = st_pool.tile([128, 1], f32,
                                               name=f"{sname}sc{l}{oc}",
                                               tag=f"sc{oc}")
                            nc.vector.memset(scl[:, :], 1.0)
                            bia = st_pool.tile([128, 1], f32,
                                               name=f"{sname}bi{l}{oc}",
                                               tag=f"bi{oc}")
                            nc.vector.memset(bia[:, :], 0.0)
                            sc_t.append(scl)
                            bi_t.append(bia)
                        pxin = newpx
                        continue
                    # cross-core AllGather + local reduce
                    cci = dr_pool.tile([128, 4], f32, name=f"{sname}cci{l}",
                                       tag="cci")
                    cco = dr_pool.tile([128 * NCORES, 4], f32,
                                       name=f"{sname}cco{l}",
                                       tag="cco", addr_space="Shared")
                    nc.sync.dma_start(out=cci[:, :], in_=ccs[:, :])
                    nc.gpsimd.collective_compute(
                        "AllGather", ALU.bypass,
                        replica_groups=[list(range(NCORES))],
                        ins=[cci.opt()], outs=[cco.opt()])
                    allg = st_pool.tile([128, 4 * NCORES], f32,
                                        name=f"{sname}allg{l}", tag="allg")
                    nc.sync.dma_start(
                        out=allg.rearrange("p (r s) -> p r s", r=NCORES, s=4),
                        in_=cco.rearrange("(r p) s -> p r s", r=NCORES, p=128))
                    gst = st_pool.tile([128, 4], f32, name=f"{sname}gst{l}",
                                       tag="gst")
                    nc.vector.tensor_reduce(
                        out=gst[:, :],
                        in_=allg.rearrange("p (r s) -> p s r", r=NCORES, s=4),
                        axis=AX.X, op=ALU.add)

                    if stats_lvl < 4:
                        sc_t, bi_t = [], []
                        for oc in range(2):
                            scl = st_pool.tile([128, 1], f32,
                                               name=f"{sname}sc{l}{oc}",
                                               tag=f"sc{oc}")
                            nc.vector.memset(scl[:, :], 1.0)
                            bia = st_pool.tile([128, 1], f32,
                                               name=f"{sname}bi{l}{oc}",
                                               tag=f"bi{oc}")
                            nc.vector.memset(bia[:, :], 0.0)
                            sc_t.append(scl)
                            bi_t.append(bia)
                        pxin = newpx
                        continue
                    # per-chunk scale/bias
                    sc_t, bi_t = [], []
                    for oc in range(2):
                        mean = st_pool.tile([128, 1], f32,
                                            name=f"{sname}mn{l}{oc}", tag=f"mn{oc}")
                        nc.scalar.mul(mean[:, :], gst[:, oc:oc + 1], 1.0 / CNT)
                        msq = st_pool.tile([128, 1], f32,
                                           name=f"{sname}ms{l}{oc}", tag=f"ms{oc}")
                        nc.scalar.square(msq[:, :], mean[:, :])
                        var = st_pool.tile([128, 1], f32,
                                           name=f"{sname}vr{l}{oc}", tag=f"vr{oc}")
                        nc.vector.scalar_tensor_tensor(
                            out=var[:, :], in0=gst[:, 2 + oc:3 + oc],
                            scalar=1.0 / CNT, in1=msq[:, :],
                            op0=ALU.mult, op1=ALU.subtract)
                        std = st_pool.tile([128, 1], f32,
                                           name=f"{sname}sd{l}{oc}", tag=f"sd{oc}")
                        nc.scalar.activation(std[:, :], var[:, :], ACT.Sqrt,
                                             bias=eps_t[:, :])
                        rstd = st_pool.tile([128, 1], f32,
                                            name=f"{sname}rs{l}{oc}", tag=f"rs{oc}")
                        nc.vector.reciprocal(rstd[:, :], std[:, :])
                        scl = st_pool.tile([128, 1], f32,
                                           name=f"{sname}sc{l}{oc}", tag=f"sc{oc}")
                        nc.vector.tensor_mul(scl[:, :], gt[oc][:, :], rstd[:, :])
                        mtmp = st_pool.tile([128, 1], f32,
                                            name=f"{sname}mt{l}{oc}", tag=f"mt{oc}")
                        nc.vector.tensor_mul(mtmp[:, :], mean[:, :], scl[:, :])
                        bia = st_pool.tile([128, 1], f32,
                                           name=f"{sname}bi{l}{oc}", tag=f"bi{oc}")
                        nc.vector.tensor_sub(bia[:, :], bt[oc][:, :], mtmp[:, :])
                        sc_t.append(scl)
                        bi_t.append(bia)

                # apply BN (+res/relu/mask) in place
                for s in range(0 if no_apply else NL):
                    for oc in range(2):
                        src = newpx[(s, oc)][:, 1:1 + T]
                        mpay = mask_c[s][:, 1:1 + T]
                        if l % 2 == 0:
                            if l < 5:
                                nc.scalar.activation(src, src, ACT.Relu,
                                                     bias=bi_t[oc][:, :],
                                                     scale=sc_t[oc][:, :])
                                nc.vector.tensor_mul(src, src, mpay)
                        elif l == 1:
                            nc.scalar.activation(src, src, ACT.Identity,
                                                 bias=bi_t[oc][:, :],
                                                 scale=sc_t[oc][:, :])
                            rt = res_pool.tile([128, T], cdt,
                                               name=f"{sname}res{s}{oc}",
                                               tag=f"res{s}{oc}")
                            nc.vector.tensor_copy(rt[:, :], src)
                            res_t[(s, oc)] = rt
                            nc.vector.scalar_tensor_tensor(
                                out=src, in0=rt[:, :], scalar=0.0, in1=mpay,
                                op0=ALU.max, op1=ALU.mult)
                        else:  # l in (3, 5)
                            nc.scalar.activation(src, src, ACT.Identity,
                                                 bias=bi_t[oc][:, :],
                                                 scale=sc_t[oc][:, :])
                            rt = res_t[(s, oc)]
                            nc.vector.tensor_add(rt[:, :], rt[:, :], src)
                            if l == 3:
                                nc.vector.scalar_tensor_tensor(
                                    out=src, in0=rt[:, :], scalar=0.0, in1=mpay,
                                    op0=ALU.max, op1=ALU.mult)
                pxin = newpx
            return res_t  # y5 == final residual tiles

        # ---------------- main stack + heads ----------------
        y5m = conv_stack("m", w0_m, w_m, bng_m, bnb_m, cdt_m)

        for s in range(NL if (do_heads and n_layers == 6) else 0):
            qs = qs_pool.tile([128, T], cdt_m, name=f"qs{s}", tag="qs")
            for t3 in range(NT3):
                ph = hd_pool.tile([128, TCH], f32, name=f"qh{s}{t3}", tag="hd")
                for ic in range(2):
                    nc.tensor.matmul(ph[:, :], lhsT=qw_t[ic][:, :],
                                     rhs=y5m[(s, ic)][:, t3 * TCH:(t3 + 1) * TCH],
                                     start=(ic == 0), stop=(ic == 1))
                nc.scalar.activation(qs[:, t3 * TCH:(t3 + 1) * TCH], ph[:, :],
                                     ACT.Identity, bias=qb_t[:, :])
            qsq = sc_pool.tile([128, T], cdt_m, name=f"qsq{s}", tag="scr")
            nc.vector.tensor_mul(qsq[:, :], qs[:, :], qs[:, :])
            qn_sb = st_pool.tile([1, T], f32, name=f"qnsb{s}", tag="qnsb")
            for t3 in range(NT3):
                pq = hd_pool.tile([1, TCH], f32, name=f"pq{s}{t3}", tag="hd")
                nc.tensor.matmul(pq[:, :], lhsT=ones_t[:, :],
                                 rhs=qsq[:, t3 * TCH:(t3 + 1) * TCH],
                                 start=True, stop=True)
                nc.scalar.copy(qn_sb[:, t3 * TCH:(t3 + 1) * TCH], pq[:, :])
            nc.sync.dma_start(out=qn2_out[s:s + 1, :], in_=qn_sb[:, :])
            for mx in range(2):
                m0, msz = (0, 128) if mx == 0 else (128, TX - 128)
                for t3 in range(NT3):
                    pd = hd_pool.tile([msz, TCH], f32, name=f"pd{s}{mx}{t3}",
                                      tag="hd")
                    nc.tensor.matmul(pd[:, :], lhsT=keyn_t[s][:, m0:m0 + msz],
                                     rhs=qs[:, t3 * TCH:(t3 + 1) * TCH],
                                     start=True, stop=True)
                    ds = sc_pool.tile([msz, TCH], f32, name=f"dsb{s}{mx}{t3}",
                                      tag="hsb", bufs=3)
                    nc.scalar.copy(ds[:, :], pd[:, :])
                    nc.sync.dma_start(
                        out=d_out[s, m0:m0 + msz, t3 * TCH:(t3 + 1) * TCH],
                        in_=ds[:, :])

        # ---------------- aux stack + ctc head ----------------
        y5a = conv_stack("a", w0_a, w_a, bng_a, bnb_a, cdt_a) \
            if n_stacks == 2 else None

        for s in range(NL if (n_stacks == 2 and do_heads and n_layers == 6) else 0):
            for t3 in range(NT3):
                pc = hd_pool.tile([V, TCH], f32, name=f"pc{s}{t3}", tag="hd")
                for ic in range(2):
                    nc.tensor.matmul(pc[:, :], lhsT=ctcw_t[ic][:, :],
                                     rhs=y5a[(s, ic)][:, t3 * TCH:(t3 + 1) * TCH],
                                     start=(ic == 0), stop=(ic == 1))
                cs = sc_pool.tile([V, TCH], f32, name=f"csb{s}{t3}",
                                  tag="hsb", bufs=3)
                nc.scalar.copy(cs[:, :], pc[:, :])
                nc.sync.dma_start(out=ctc_out[s, :, t3 * TCH:(t3 + 1) * TCH],
                                  in_=cs[:, :])

    nc.compile()
    return nc


# ---------------------------------------------------------------------------
# host side
# ---------------------------------------------------------------------------

def _prep(text, spec, text_lengths, spec_lengths, text_mask, short_token_mask,
          params):
    """Build per-core device inputs + host context."""
    text = np.asarray(text).astype(np.int64)
    spec = np.asarray(spec, np.float32)
    text_lengths = np.asarray(text_lengths).astype(np.int64)
    spec_lengths = np.asarray(spec_lengths).astype(np.int64)
    text_mask = np.asarray(text_mask, np.float32)
    stm = np.asarray(short_token_mask, np.float32)

    p = {k: params[k] for k in params}
    conv_w = [np.asarray(w, np.float32) for w in p['conv_w']]
    conv_wa = [np.asarray(w, np.float32) for w in p['conv_w_aux']]
    bng_m = np.stack([np.asarray(g, np.float32) for g in p['bn_g']])
    bnb_m = np.stack([np.asarray(b, np.float32) for b in p['bn_b']])
    bng_a = np.stack([np.asarray(g, np.float32) for g in p['bn_g_aux']])
    bnb_a = np.stack([np.asarray(b, np.float32) for b in p['bn_b_aux']])
    emb = np.asarray(p['emb'], np.float32)
    q_w = np.asarray(p['q_w'], np.float32)
    q_b = np.asarray(p['q_b'], np.float32)
    ctc_w = np.asarray(p['ctc_w'], np.float32)
    ctc_b = np.asarray(p['ctc_b'], np.float32)
    sim_w = np.asarray(p['sim_w'], np.float32)
    sim_b = np.asarray(p['sim_b'], np.float32)

    smask = (np.arange(T)[None] < spec_lengths[:, None]).astype(np.float32)
    spec_t = spec.transpose(0, 2, 1) * smask[:, None, :]
    spec_pm = np.zeros((N, MEL, TP), np.float32)
    spec_pm[:, :, 1:1 + T] = spec_t
    mask_b = np.zeros((N, 128, TP), np.float32)
    mask_b[:, :, 1:1 + T] = smask[:, None, :]

    key = emb[text] * text_mask[:, :, None]
    keyn = key / np.maximum(
        np.linalg.norm(key, axis=2, keepdims=True).astype(np.float32), 1e-8)
    keynT = np.ascontiguousarray(keyn.transpose(0, 2, 1), np.float32)

    w0_m = np.ascontiguousarray(
        np.stack([conv_w[0][:, :, k].T for k in range(3)]), np.float32)
    w_m = np.ascontiguousarray(
        np.stack([np.stack([conv_w[l][:, :, k].T for k in range(3)])
                  for l in range(1, 6)]), np.float32)
    w0_a = np.ascontiguousarray(
        np.stack([conv_wa[0][:, :, k].T for k in range(3)]), np.float32)
    w_a = np.ascontiguousarray(
        np.stack([np.stack([conv_wa[l][:, :, k].T for k in range(3)])
                  for l in range(1, 6)]), np.float32)

    shared = dict(
        w0_m=w0_m, w_m=w_m, w0_a=w0_a, w_a=w_a,
        bng_m=np.ascontiguousarray(bng_m[:, :, None], np.float32),
        bnb_m=np.ascontiguousarray(bnb_m[:, :, None], np.float32),
        bng_a=np.ascontiguousarray(bng_a[:, :, None], np.float32),
        bnb_a=np.ascontiguousarray(bnb_a[:, :, None], np.float32),
        qwT=np.ascontiguousarray(q_w.T, np.float32),
        qb=np.ascontiguousarray(q_b[:, None], np.float32),
        ctcwT=np.ascontiguousarray(ctc_w.T, np.float32),
    )
    in_maps = []
    for i in range(NCORES):
        sl = slice(i * NL, (i + 1) * NL)
        m = dict(shared)
        m['spec_pm'] = np.ascontiguousarray(spec_pm[sl])
        m['mask_b'] = np.ascontiguousarray(mask_b[sl])
        m['keynT'] = np.ascontiguousarray(keynT[sl])
        in_maps.append(m)

    host = dict(text=text, spec=spec, text_lengths=text_lengths,
                spec_lengths=spec_lengths, text_mask=text_mask, stm=stm,
                ctc_b=ctc_b, sim_w=sim_w, sim_b=sim_b)
    return in_maps, host


def _log_sigmoid(x):
    return -np.logaddexp(np.float32(0.0), -x)


def _maximum_path(value, t_x, t_y):
    """numpy port of the Glow-TTS monotonic alignment search (f32)."""
    Nb, Tx, Ty = value.shape
    xs = np.arange(Tx)[None]
    dp = np.full((Nb, Tx), NEG, np.float32)
    dp_table = np.empty((Nb, Tx, Ty), np.float32)
    tx = t_x[:, None]
    ty = t_y[:, None]
    for y in range(Ty):
        v = value[:, :, y]
        shifted = np.concatenate(
            [np.full((Nb, 1), NEG, np.float32), dp[:, :-1]], axis=1)
        best = np.maximum(dp, shifted)
        if y == 0:
            base = np.where(xs == 0, np.float32(0.0), np.float32(NEG))
        else:
            base = best
        dp_new = v + base
        valid = (xs <= y) & (xs >= tx + y - ty) & (xs < tx) & (y < ty)
        dp = np.where(valid, dp_new, np.float32(NEG)).astype(np.float32)
        dp_table[:, :, y] = dp
    bi = np.arange(Nb)
    idx = (t_x - 1).astype(np.int64)
    path = np.zeros((Nb, Tx, Ty), np.float32)
    for y in range(Ty - 1, -1, -1):
        active = y < t_y
        col_prev = dp_table[:, :, max(y - 1, 0)]
        v_cur = col_prev[bi, idx]
        v_prev = col_prev[bi, np.maximum(idx - 1, 0)]
        path[bi, idx, y] = active.astype(np.float32)
        move = active & (idx > 0) & (y > 0) & ((idx == y) | (v_cur < v_prev))
        idx = idx - move.astype(np.int64)
    return path


def _ctc_loss_mean(log_probs, targets, in_lens, tgt_lens):
    """numpy port of the reference CTC loss (f32)."""
    Tt, Nb, Cc = log_probs.shape
    S = targets.shape[1]
    L = 2 * S + 1
    z = np.zeros((Nb, L), targets.dtype)
    z[:, 1::2] = targets
    sidx = np.arange(L)[None]
    z_m2 = np.pad(z, ((0, 0), (2, 0)))[:, :L]
    skip = (sidx % 2 == 1) & (sidx >= 2) & (z != z_m2)

    e0 = np.take_along_axis(log_probs[0], z, axis=1)
    alpha = np.full((Nb, L), NEG, np.float32)
    alpha[:, 0] = e0[:, 0]
    alpha[:, 1] = e0[:, 1]
    negpad = np.full((Nb, 1), NEG, np.float32)
    negpad2 = np.full((Nb, 2), NEG, np.float32)
    for t in range(1, Tt):
        lp = log_probs[t]
        a2 = np.concatenate([negpad, alpha[:, :-1]], axis=1)
        a3 = np.where(skip, np.concatenate([negpad2, alpha[:, :-2]], axis=1),
                      np.float32(NEG))
        new = np.take_along_axis(lp, z, axis=1) + np.logaddexp(
            np.logaddexp(alpha, a2), a3)
        alpha = np.where((t < in_lens)[:, None], new, alpha).astype(np.float32)
    bi = np.arange(Nb)
    ll = np.logaddexp(alpha[bi, 2 * tgt_lens], alpha[bi, 2 * tgt_lens - 1])
    return np.mean(-ll / tgt_lens.astype(ll.dtype))


def _finish(D, qn2, ctcT, host):
    """Everything after the device part. D [N,TX,T]; qn2 [N,T]; ctcT [N,V,T]."""
    text = host['text']
    spec = host['spec']
    text_lengths = host['text_lengths']
    spec_lengths = host['spec_lengths']
    stm = host['stm']

    qnorm = np.sqrt(np.maximum(qn2, 0.0)).astype(np.float32)
    cos = D / np.maximum(qnorm, 1e-8)[:, None, :]
    cos = (1.0 - stm) * cos - stm
    similarity = (10.0 * np.exp(host['sim_w']) * cos + host['sim_b']).astype(
        np.float32)

    ctc_logit = ctcT.transpose(0, 2, 1) + host['ctc_b']  # [N, T, V]
    # softmax over V
    mx = ctc_logit.max(axis=2, keepdims=True)
    ex = np.exp(ctc_logit - mx)
    ctc_query = (ex / ex.sum(axis=2, keepdims=True)).astype(np.float32)
    sim_ctc = np.take_along_axis(
        ctc_query.transpose(0, 2, 1), text[:, :, None], axis=1).astype(np.float32)

    tm_b = np.arange(TX)[None] < text_lengths[:, None]
    sm_b = np.arange(T)[None] < spec_lengths[:, None]
    att_mask = (tm_b[:, :, None] & sm_b[:, None, :]).astype(np.float32)

    logsig = _log_sigmoid(similarity).astype(np.float32)
    lsmx = logsig * att_mask
    lsmx_att = lsmx - (lsmx == 0).astype(np.float32) * lsmx.min()
    match_mask = _maximum_path(lsmx_att, text_lengths, spec_lengths)
    attention = match_mask

    lsmx2 = (_log_sigmoid(sim_ctc) * att_mask).astype(np.float32)
    lsmx_aux = lsmx2 - (lsmx2 == 0).astype(np.float32) * lsmx2.min()
    att_aux = _maximum_path(lsmx_aux, text_lengths, spec_lengths)
    aa = np.pad(att_aux, ((0, 0), (1, 1), (0, 0)))
    att_aux = ((aa[:, :-2] + aa[:, 1:-1] + aa[:, 2:]) * att_mask).astype(
        np.float32)

    neg_logsig = _log_sigmoid(-similarity).astype(np.float32)
    denom = att_mask.sum(axis=(1, 2))
    inter = -(match_mask * logsig + (1 - match_mask) * att_mask * neg_logsig)
    nll = np.float32(np.mean(inter.sum(axis=(1, 2)) / denom))
    att_loss = nll
    aux_l = -(att_aux * logsig + (1 - att_aux) * att_mask * neg_logsig)
    att_loss = att_loss + np.float32(
        np.mean(aux_l.sum(axis=(1, 2)) / denom * 0.5))

    # CTC
    lmx = ctc_logit.max(axis=2, keepdims=True)
    lse = lmx + np.log(np.exp(ctc_logit - lmx).sum(axis=2, keepdims=True))
    ctc_in = np.ascontiguousarray(
        (ctc_logit - lse).transpose(1, 0, 2), np.float32)  # [T, N, V]
    att_loss = att_loss + np.float32(
        _ctc_loss_mean(ctc_in, text, spec_lengths, text_lengths))

    # silence promotion
    tm_i = tm_b.astype(np.int32)
    sm_sil = tm_i.copy()
    sm_sil[:, :-1] += -tm_i[:, 1:]
    sm_sil[:, 0] = 1
    silence = sm_sil[:, :, None].astype(np.float32)
    energy = np.mean(np.exp(spec[:, :, 20:]), axis=2)[:, None, :].astype(
        np.float32)
    se_max = np.max(np.sum(energy * silence * attention, axis=1), axis=1)
    ns_min = np.sum(energy * (1 - silence) * attention, axis=1)
    ns_min = np.min((ns_min == 0).astype(np.float32) * 100 + ns_min, axis=1)
    db = ((se_max + ns_min) / 2)[:, None, None]
    promo = (energy <= db).astype(np.float32) * silence
    sp = -0.01 * np.sum(promo * att_mask * logsig, axis=(1, 2)) / np.maximum(
        np.sum(promo * att_mask, axis=(1, 2)), 1.0)
    att_loss = att_loss + np.float32(np.mean(sp))

    return (attention.astype(np.float32), np.float32(att_loss),
            att_mask, np.float32(nll))


# ---------------------------------------------------------------------------
# numpy twin of the device math (for validation without hardware)
# ---------------------------------------------------------------------------

def _device_twin(in_maps):
    """Replicates the device computation in numpy at full-batch level."""
    spec_pm = np.concatenate([m['spec_pm'] for m in in_maps])   # [N, MEL, TP]
    mask_b = np.concatenate([m['mask_b'] for m in in_maps])
    keynT = np.concatenate([m['keynT'] for m in in_maps])
    sh = in_maps[0]

    def stack(w0, w, g, b):
        px = spec_pm.copy()  # [N, K, TP]
        res = None
        for l in range(6):
            wk = [w0[k] for k in range(3)] if l == 0 else \
                 [w[l - 1, k] for k in range(3)]
            conv = np.zeros((N, C, T), np.float32)
            for k in range(3):
                # out[:, oc, t] += wk[k].T @ px[:, ic, t+k]
                conv += np.matmul(wk[k].T[None], px[:, :, k:k + T])
            s1 = conv.sum(axis=(0, 2))
            s2 = (conv * conv).sum(axis=(0, 2))
            mean = s1 / CNT
            var = s2 / CNT - mean * mean
            scl = g[l, :, 0] / np.sqrt(var + EPS)
            bia = b[l, :, 0] - mean * scl
            y = conv * scl[None, :, None] + bia[None, :, None]
            if l % 2 == 1:
                res = y if res is None else (y + res)
                y = res
            if l < 5:
                nxt = np.zeros((N, C, TP), np.float32)
                nxt[:, :, 1:1 + T] = np.maximum(y, 0.0) * mask_b[:, 0:1, 1:1 + T]
                px = nxt
            else:
                return y
        return None

    y5m = stack(sh['w0_m'], sh['w_m'], sh['bng_m'], sh['bnb_m'])
    y5a = stack(sh['w0_a'], sh['w_a'], sh['bng_a'], sh['bnb_a'])

    qT = np.matmul(sh['qwT'].T[None], y5m) + sh['qb'][None, :, :]
    qn2 = (qT * qT).sum(axis=1)
    D = np.matmul(keynT.transpose(0, 2, 1), qT)
    ctcT = np.matmul(sh['ctcwT'].T[None], y5a)
    return D, qn2, ctcT


# ---------------------------------------------------------------------------
# entry point
# ---------------------------------------------------------------------------

def kernel(**inputs):
    global _NC
    in_maps, host = _prep(**inputs)

    if os.environ.get("KERNEL_TWIN"):
        D, qn2, ctcT = _device_twin(in_maps)
        return _finish(D, qn2, ctcT, host)

    from concourse.bass_utils import run_bass_kernel_spmd
    if _NC is None:
        _NC = _build_nc()
    res = run_bass_kernel_spmd(_NC, in_maps, core_ids=list(range(NCORES)))
    outs = res.results
    D = np.concatenate([np.asarray(o['d_out']) for o in outs])
    qn2 = np.concatenate([np.asarray(o['qn2_out']) for o in outs])
    ctcT = np.concatenate([np.asarray(o['ctc_out']) for o in outs])
    return _finish(D, qn2, ctcT, host)


# revision 33
# speedup vs baseline: 1.3390x; 1.0362x over previous
"""Trainium2 Bass kernel for nn_Attention_40767829574409.

Data-parallel over batch N=32 across 8 NeuronCores (4 samples/core).
Device computes (per core, per sample):
  - two 6-layer conv(3,pad1)+BN stacks (BN stats exact via AllGather of
    per-core partial sums across the 8 cores), residual every odd layer,
    relu except last, per-layer length masking
  - query head:  qT = q_w @ y5 + q_b ; qn2[t] = sum_h qT[h,t]^2
  - cosine numerator D[x,t] = keyn[x,:] @ qT[:,t]  (keyn host-normalized)
  - ctc head: ctc_logitT = ctc_w @ y5_aux
Host computes the cheap/sequential remainder: cosine division, similarity,
log-sigmoid losses, monotonic-alignment DP (maximum_path), CTC loss,
silence-promotion loss. Returns (attention, att_loss, att_mask, nll).
"""

import os
import sys

import numpy as np

for _p in ("/opt/trn_rl_repo", "/opt/trn_rl_repo/concourse"):
    if _p not in sys.path:
        sys.path.insert(0, _p)

# problem constants (hardcoded per spec)
NCORES = 8
N = 32
NL = N // NCORES          # samples per core
T = 1200                  # T_DEC
TP = T + 2                # padded time axis (zero col at 0 and T+1)
TX = 240                  # T_TEXT
MEL = 80
HID = 128
C = 256                   # ENC_HID
V = 100                   # VOCAB
TCH = 400                 # matmul free-dim chunk
NT3 = T // TCH            # 3 chunks
CNT = float(N * T)        # BatchNorm element count per channel
EPS = 1e-5
NEG = -1e9

_NC = None                # cached Bass graph


# ---------------------------------------------------------------------------
# device graph
# ---------------------------------------------------------------------------

def _build_nc():
    import concourse.bacc as bacc
    import concourse.tile as tile
    from concourse import mybir
    from contextlib import ExitStack

    f32 = mybir.dt.float32
    ALU = mybir.AluOpType
    ACT = mybir.ActivationFunctionType
    AX = mybir.AxisListType

    nc = bacc.Bacc("TRN2", target_bir_lowering=False, num_devices=NCORES)

    # ---- I/O ----
    spec_pm = nc.declare_dram_parameter("spec_pm", [NL, MEL, TP], f32, False)
    mask_b = nc.declare_dram_parameter("mask_b", [NL, 128, TP], f32, False)
    keynT = nc.declare_dram_parameter("keynT", [NL, HID, TX], f32, False)
    w0_m = nc.declare_dram_parameter("w0_m", [3, MEL, C], f32, False)
    w_m = nc.declare_dram_parameter("w_m", [5, 3, C, C], f32, False)
    w0_a = nc.declare_dram_parameter("w0_a", [3, MEL, C], f32, False)
    w_a = nc.declare_dram_parameter("w_a", [5, 3, C, C], f32, False)
    bng_m = nc.declare_dram_parameter("bng_m", [6, C, 1], f32, False)
    bnb_m = nc.declare_dram_parameter("bnb_m", [6, C, 1], f32, False)
    bng_a = nc.declare_dram_parameter("bng_a", [6, C, 1], f32, False)
    bnb_a = nc.declare_dram_parameter("bnb_a", [6, C, 1], f32, False)
    qwT = nc.declare_dram_parameter("qwT", [C, HID], f32, False)
    qb = nc.declare_dram_parameter("qb", [HID, 1], f32, False)
    ctcwT = nc.declare_dram_parameter("ctcwT", [C, V], f32, False)

    d_out = nc.declare_dram_parameter("d_out", [NL, TX, T], f32, True)
    qn2_out = nc.declare_dram_parameter("qn2_out", [NL, T], f32, True)
    ctc_out = nc.declare_dram_parameter("ctc_out", [NL, V, T], f32, True)

    with tile.TileContext(nc) as tc, ExitStack() as ctx:
        const_pool = ctx.enter_context(tc.tile_pool(name="const", bufs=1))
        mask_pool = ctx.enter_context(tc.tile_pool(name="maskp", bufs=1))
        px_pool = ctx.enter_context(tc.tile_pool(name="px", bufs=2))
        res_pool = ctx.enter_context(tc.tile_pool(name="res", bufs=1))
        w_pool = ctx.enter_context(tc.tile_pool(name="wp", bufs=2))
        sc_pool = ctx.enter_context(tc.tile_pool(name="scr", bufs=2))
        st_pool = ctx.enter_context(tc.tile_pool(name="st", bufs=2))
        qs_pool = ctx.enter_context(tc.tile_pool(name="qs", bufs=2))
        ps_pool = ctx.enter_context(tc.tile_pool(name="ps", bufs=6, space="PSUM"))
        hd_pool = ctx.enter_context(tc.tile_pool(name="hd", bufs=2, space="PSUM"))
        dr_pool = ctx.enter_context(tc.tile_pool(name="dr", bufs=2, space="DRAM"))

        # ---- resident constants ----
        # Masks/keyn/head weights go through gpsimd DMA queues so they do not
        # queue ahead of the conv weights on the sync (HWDGE) engine.
        mask_t = []
        for s in range(NL):
            mt = mask_pool.tile([128, TP], f32, name=f"mask{s}", tag=f"mask{s}")
            nc.gpsimd.dma_start(out=mt[:, :], in_=mask_b[s])
            mask_t.append(mt)
        keyn_t = []
        for s in range(NL):
            kt = const_pool.tile([HID, TX], f32, name=f"keyn{s}", tag=f"keyn{s}")
            nc.gpsimd.dma_start(out=kt[:, :], in_=keynT[s])
            keyn_t.append(kt)
        qw_t = []
        for ic in range(2):
            qt = const_pool.tile([128, HID], f32, name=f"qw{ic}", tag=f"qw{ic}")
            nc.gpsimd.dma_start(out=qt[:, :], in_=qwT[ic * 128:(ic + 1) * 128, :])
            qw_t.append(qt)
        qb_t = const_pool.tile([HID, 1], f32, name="qbt", tag="qbt")
        nc.gpsimd.dma_start(out=qb_t[:, :], in_=qb[:, :])
        ctcw_t = []
        for ic in range(2):
            ct = const_pool.tile([128, V], f32, name=f"ctcw{ic}", tag=f"ctcw{ic}")
            nc.gpsimd.dma_start(out=ct[:, :],
                                in_=ctcwT[ic * 128:(ic + 1) * 128, :])
            ctcw_t.append(ct)
        ones_t = const_pool.tile([128, 1], f32, name="ones", tag="ones")
        nc.vector.memset(ones_t[:, :], 1.0)
        eps_t = const_pool.tile([128, 2], f32, name="epst", tag="epst")
        nc.vector.memset(eps_t[:, :], EPS)

        n_layers = int(os.environ.get("KB_LAYERS", "6"))
        n_stacks = int(os.environ.get("KB_STACKS", "2"))
        do_heads = os.environ.get("KB_HEADS", "1") == "1"
        no_stats = os.environ.get("KB_NOSTATS") == "1"
        stats_lvl = int(os.environ.get("KB_STATS_LVL", "4"))
        no_acc = os.environ.get("KB_NOACC") == "1"
        no_apply = os.environ.get("KB_NOAPPLY") == "1"
        no_memset = os.environ.get("KB_NOMEMSET") == "1"

        dtmap = {"f32": f32, "f32r": mybir.dt.float32r,
                 "bf16": mybir.dt.bfloat16}
        cdt_m = dtmap[os.environ.get("KB_MMDT_MAIN", "f32")]
        cdt_a = dtmap[os.environ.get("KB_MMDT_AUX", "f32")]

        # masks converted per conv dtype (DMA cannot round to f32r/bf16)
        mask_cv = {f32: mask_t}
        for dt_ in {cdt_m, cdt_a} - {f32}:
            cvl = []
            for s in range(NL):
                mc = mask_pool.tile([128, TP], dt_,
                                    name=f"maskc{dt_.value}{s}",
                                    tag=f"maskc{dt_.value}{s}")
                nc.vector.tensor_copy(mc[:, :], mask_t[s][:, :])
                cvl.append(mc)
            mask_cv[dt_] = cvl

        # head weights converted to the owning stack's dtype
        def conv_consts(tiles, dt_, prefix):
            if dt_ == f32:
                return tiles
            out = []
            for i, tl in enumerate(tiles):
                cc = const_pool.tile(list(tl.shape), dt_,
                                     name=f"{prefix}{i}c", tag=f"{prefix}{i}c")
                nc.vector.tensor_copy(cc[:, :], tl[:, :])
                out.append(cc)
            return out

        qw_t = conv_consts(qw_t, cdt_m, "qwc")
        keyn_t = conv_consts(keyn_t, cdt_m, "knc")
        ctcw_t = conv_consts(ctcw_t, cdt_a, "ctc")
        ones_t = conv_consts([ones_t], cdt_m, "one")[0]

        def conv_stack(sname, w0_h, w_h, g_h, b_h, cdt):
            """Runs the 6-layer stack; returns {(s, oc): y5 AP [128, T]}."""
            mask_c = mask_cv[cdt]
            # layer-0 inputs
            pxin = {}
            for s in range(NL):
                p0 = px_pool.tile([MEL, TP], cdt, name=f"{sname}px0_{s}",
                                  tag=f"px_{s}_0")
                if cdt == f32:
                    nc.sync.dma_start(out=p0[:, :], in_=spec_pm[s])
                else:
                    p0f = sc_pool.tile([MEL, TP], f32, name=f"{sname}px0f{s}",
                                       tag="scr")
                    nc.sync.dma_start(out=p0f[:, :], in_=spec_pm[s])
                    nc.vector.tensor_copy(p0[:, :], p0f[:, :])
                pxin[(s, 0)] = p0
            res_t = {}

            for l in range(n_layers):
                nic = 1 if l == 0 else 2
                # weights
                wt = {}
                for k3 in range(3):
                    for ic in range(nic):
                        kdim = MEL if l == 0 else 128
                        src = w0_h[k3] if l == 0 else \
                            w_h[l - 1, k3, ic * 128:(ic + 1) * 128, :]
                        if cdt == f32:
                            w = w_pool.tile([kdim, C], f32,
                                            name=f"{sname}w{l}_{k3}_{ic}",
                                            tag=f"w{k3}{ic}")
                            nc.sync.dma_start(out=w[:, :], in_=src)
                        else:
                            wf = w_pool.tile([kdim, C], f32,
                                             name=f"{sname}wf{l}_{k3}_{ic}",
                                             tag=f"wf{k3}{ic}")
                            nc.sync.dma_start(out=wf[:, :], in_=src)
                            w = w_pool.tile([kdim, C], cdt,
                                            name=f"{sname}w{l}_{k3}_{ic}",
                                            tag=f"w{k3}{ic}")
                            nc.vector.tensor_copy(w[:, :], wf[:, :])
                        wt[(k3, ic)] = w
                # bn params as [128, 2] (col per channel chunk)
                g2 = st_pool.tile([128, 2], f32, name=f"{sname}g{l}", tag="g2")
                nc.gpsimd.dma_start(
                    out=g2[:, :],
                    in_=g_h[l].rearrange("(o p) one -> p (o one)", o=2, p=128))
                b2 = st_pool.tile([128, 2], f32, name=f"{sname}b{l}", tag="b2")
                nc.gpsimd.dma_start(
                    out=b2[:, :],
                    in_=b_h[l].rearrange("(o p) one -> p (o one)", o=2, p=128))

                # partial stats: cols 0..23 sums per (s,oc,t3); 24..31 sq per (s,oc)
                pt = st_pool.tile([128, 32], f32, name=f"{sname}pt{l}", tag="pt")

                newpx = {}
                for s in range(NL):
                    for oc in range(2):
                        newpx[(s, oc)] = px_pool.tile(
                            [128, TP], cdt, name=f"{sname}px{l + 1}_{s}_{oc}",
                            tag=f"px_{s}_{oc}")
                        nc.gpsimd.memset(newpx[(s, oc)][:, 0:1], 0.0)
                        nc.gpsimd.memset(newpx[(s, oc)][:, TP - 1:TP], 0.0)
                for s in range(NL):
                    for oc in range(2):
                        tnew = newpx[(s, oc)]
                        psums = [ps_pool.tile([128, TCH], f32,
                                              name=f"{sname}ps{l}{s}{oc}{t3}",
                                              tag="cv") for t3 in range(NT3)]
                        # weight-stationary: same lhsT feeds all 3 t-chunks
                        for ic in range(nic):
                            for k3 in range(3):
                                first = (ic == 0 and k3 == 0)
                                last = (ic == nic - 1 and k3 == 2)
                                for t3 in range(NT3):
                                    nc.tensor.matmul(
                                        psums[t3][:, :],
                                        lhsT=wt[(k3, ic)][:, oc * 128:(oc + 1) * 128],
                                        rhs=pxin[(s, ic)][:, t3 * TCH + k3:
                                                          t3 * TCH + k3 + TCH],
                                        start=first, stop=last)
                        for t3 in range(NT3):
                            # copy PSUM -> px payload, accumulate row-sum
                            col = s * 6 + oc * 3 + t3
                            nc.vector.tensor_scalar(
                                out=tnew[:, 1 + t3 * TCH:1 + (t3 + 1) * TCH],
                                in0=psums[t3][:, :], scalar1=0.0, scalar2=None,
                                op0=ALU.add, op1=ALU.add,
                                accum_out=pt[:, col:col + 1])
                        # sum of squares over the whole payload
                        scr = sc_pool.tile([128, T], f32,
                                           name=f"{sname}sq{l}{s}{oc}", tag="scr")
                        sqcol = 24 + s * 2 + oc
                        nc.scalar.activation(
                            scr[:, :], tnew[:, 1:1 + T], ACT.Square,
                            accum_out=pt[:, sqcol:sqcol + 1])

                # finalize local stats -> ccs [128,4] = (s0, s1, sq0, sq1)
                ccs = st_pool.tile([128, 4], f32, name=f"{sname}ccs{l}",
                                   tag="ccs")
                nc.vector.tensor_reduce(
                    out=ccs[:, 0:2],
                    in_=pt[:, 0:24].rearrange("p (s o t) -> p o s t",
                                              s=NL, o=2, t=3),
                    axis=AX.XY, op=ALU.add)
                nc.vector.tensor_reduce(
                    out=ccs[:, 2:4],
                    in_=pt[:, 24:32].rearrange("p (s o) -> p o s", s=NL, o=2),
                    axis=AX.X, op=ALU.add)

                # cross-core AllGather + local reduce
                cci = dr_pool.tile([128, 4], f32, name=f"{sname}cci{l}",
                                   tag="cci")
                cco = dr_pool.tile([128 * NCORES, 4], f32,
                                   name=f"{sname}cco{l}",
                                   tag="cco", addr_space="Shared")
                nc.sync.dma_start(out=cci[:, :], in_=ccs[:, :])
                nc.gpsimd.collective_compute(
                    "AllGather", ALU.bypass,
                    replica_groups=[list(range(NCORES))],
                    ins=[cci.opt()], outs=[cco.opt()])
                allg = st_pool.tile([128, 4 * NCORES], f32,
                                    name=f"{sname}allg{l}", tag="allg")
                nc.sync.dma_start(
                    out=allg.rearrange("p (r s) -> p r s", r=NCORES, s=4),
                    in_=cco.rearrange("(r p) s -> p r s", r=NCORES, p=128))
                gst = st_pool.tile([128, 4], f32, name=f"{sname}gst{l}",
                                   tag="gst")
                nc.vector.tensor_reduce(
                    out=gst[:, :],
                    in_=allg.rearrange("p (r s) -> p s r", r=NCORES, s=4),
                    axis=AX.X, op=ALU.add)

                # scale/bias for both chunks in one [128, 2] chain
                mean2 = st_pool.tile([128, 2], f32, name=f"{sname}mn{l}",
                                     tag="mn2")
                nc.scalar.mul(mean2[:, :], gst[:, 0:2], 1.0 / CNT)
                msq2 = st_pool.tile([128, 2], f32, name=f"{sname}ms{l}",
                                    tag="ms2")
                nc.scalar.square(msq2[:, :], mean2[:, :])
                var2 = st_pool.tile([128, 2], f32, name=f"{sname}vr{l}",
                                    tag="vr2")
                nc.vector.scalar_tensor_tensor(
                    out=var2[:, :], in0=gst[:, 2:4], scalar=1.0 / CNT,
                    in1=msq2[:, :], op0=ALU.mult, op1=ALU.subtract)
                ve2 = st_pool.tile([128, 2], f32, name=f"{sname}ve{l}",
                                   tag="ve2")
                nc.vector.tensor_scalar(out=ve2[:, :], in0=var2[:, :],
                                        scalar1=float(EPS), scalar2=None,
                                        op0=ALU.add)
                std2 = st_pool.tile([128, 2], f32, name=f"{sname}sd{l}",
                                    tag="sd2")
                nc.scalar.sqrt(std2[:, :], ve2[:, :])
                rstd2 = st_pool.tile([128, 2], f32, name=f"{sname}rs{l}",
                                     tag="rs2")
                nc.vector.reciprocal(rstd2[:, :], std2[:, :])
                scl2 = st_pool.tile([128, 2], f32, name=f"{sname}sc{l}",
                                    tag="sc2")
                nc.vector.tensor_mul(scl2[:, :], g2[:, :], rstd2[:, :])
                mtmp2 = st_pool.tile([128, 2], f32, name=f"{sname}mt{l}",
                                     tag="mt2")
                nc.vector.tensor_mul(mtmp2[:, :], mean2[:, :], scl2[:, :])
                bia2 = st_pool.tile([128, 2], f32, name=f"{sname}bi{l}",
                                    tag="bi2")
                nc.vector.tensor_sub(bia2[:, :], b2[:, :], mtmp2[:, :])
                sc_t = [scl2[:, 0:1], scl2[:, 1:2]]
                bi_t = [bia2[:, 0:1], bia2[:, 1:2]]

                # apply BN (+res/relu/mask) in place
                for s in range(0 if no_apply else NL):
                    for oc in range(2):
                        src = newpx[(s, oc)][:, 1:1 + T]
                        mpay = mask_c[s][:, 1:1 + T]
                        if l % 2 == 0:
                            if l < 5:
                                nc.scalar.activation(src, src, ACT.Relu,
                                                     bias=bi_t[oc][:, :],
                                                     scale=sc_t[oc][:, :])
                                nc.vector.tensor_mul(src, src, mpay)
                        elif l == 1:
                            nc.scalar.activation(src, src, ACT.Identity,
                                                 bias=bi_t[oc][:, :],
                                                 scale=sc_t[oc][:, :])
                            rt = res_pool.tile([128, T], cdt,
                                               name=f"{sname}res{s}{oc}",
                                               tag=f"res{s}{oc}")
                            nc.vector.tensor_copy(rt[:, :], src)
                            res_t[(s, oc)] = rt
                            nc.vector.scalar_tensor_tensor(
                                out=src, in0=rt[:, :], scalar=0.0, in1=mpay,
                                op0=ALU.max, op1=ALU.mult)
                        else:  # l in (3, 5)
                            nc.scalar.activation(src, src, ACT.Identity,
                                                 bias=bi_t[oc][:, :],
                                                 scale=sc_t[oc][:, :])
                            rt = res_t[(s, oc)]
                            nc.vector.tensor_add(rt[:, :], rt[:, :], src)
                            if l == 3:
                                nc.vector.scalar_tensor_tensor(
                                    out=src, in0=rt[:, :], scalar=0.0, in1=mpay,
                                    op0=ALU.max, op1=ALU.mult)
                pxin = newpx
            return res_t  # y5 == final residual tiles

        # ---------------- main stack + heads ----------------
        y5m = conv_stack("m", w0_m, w_m, bng_m, bnb_m, cdt_m)

        for s in range(NL if (do_heads and n_layers == 6) else 0):
            qs = qs_pool.tile([128, T], cdt_m, name=f"qs{s}", tag="qs")
            for t3 in range(NT3):
                ph = hd_pool.tile([128, TCH], f32, name=f"qh{s}{t3}", tag="hd")
                for ic in range(2):
                    nc.tensor.matmul(ph[:, :], lhsT=qw_t[ic][:, :],
                                     rhs=y5m[(s, ic)][:, t3 * TCH:(t3 + 1) * TCH],
                                     start=(ic == 0), stop=(ic == 1))
                nc.scalar.activation(qs[:, t3 * TCH:(t3 + 1) * TCH], ph[:, :],
                                     ACT.Identity, bias=qb_t[:, :])
            qsq = sc_pool.tile([128, T], cdt_m, name=f"qsq{s}", tag="scr")
            nc.vector.tensor_mul(qsq[:, :], qs[:, :], qs[:, :])
            qn_sb = st_pool.tile([1, T], f32, name=f"qnsb{s}", tag="qnsb")
            for t3 in range(NT3):
                pq = hd_pool.tile([1, TCH], f32, name=f"pq{s}{t3}", tag="hd")
                nc.tensor.matmul(pq[:, :], lhsT=ones_t[:, :],
                                 rhs=qsq[:, t3 * TCH:(t3 + 1) * TCH],
                                 start=True, stop=True)
                nc.scalar.copy(qn_sb[:, t3 * TCH:(t3 + 1) * TCH], pq[:, :])
            nc.sync.dma_start(out=qn2_out[s:s + 1, :], in_=qn_sb[:, :])
            for mx in range(2):
                m0, msz = (0, 128) if mx == 0 else (128, TX - 128)
                for t3 in range(NT3):
                    pd = hd_pool.tile([msz, TCH], f32, name=f"pd{s}{mx}{t3}",
                                      tag="hd")
                    nc.tensor.matmul(pd[:, :], lhsT=keyn_t[s][:, m0:m0 + msz],
                                     rhs=qs[:, t3 * TCH:(t3 + 1) * TCH],
                                     start=True, stop=True)
                    ds = sc_pool.tile([msz, TCH], f32, name=f"dsb{s}{mx}{t3}",
                                      tag="hsb", bufs=3)
                    nc.scalar.copy(ds[:, :], pd[:, :])
                    nc.sync.dma_start(
                        out=d_out[s, m0:m0 + msz, t3 * TCH:(t3 + 1) * TCH],
                        in_=ds[:, :])

        # ---------------- aux stack + ctc head ----------------
        y5a = conv_stack("a", w0_a, w_a, bng_a, bnb_a, cdt_a) \
            if n_stacks == 2 else None

        for s in range(NL if (n_stacks == 2 and do_heads and n_layers == 6) else 0):
            for t3 in range(NT3):
                pc = hd_pool.tile([V, TCH], f32, name=f"pc{s}{t3}", tag="hd")
                for ic in range(2):
                    nc.tensor.matmul(pc[:, :], lhsT=ctcw_t[ic][:, :],
                                     rhs=y5a[(s, ic)][:, t3 * TCH:(t3 + 1) * TCH],
                                     start=(ic == 0), stop=(ic == 1))
                cs = sc_pool.tile([V, TCH], f32, name=f"csb{s}{t3}",
                                  tag="hsb", bufs=3)
                nc.scalar.copy(cs[:, :], pc[:, :])
                nc.sync.dma_start(out=ctc_out[s, :, t3 * TCH:(t3 + 1) * TCH],
                                  in_=cs[:, :])

    nc.compile()
    return nc


# ---------------------------------------------------------------------------
# host side
# ---------------------------------------------------------------------------

def _prep(text, spec, text_lengths, spec_lengths, text_mask, short_token_mask,
          params):
    """Build per-core device inputs + host context."""
    text = np.asarray(text).astype(np.int64)
    spec = np.asarray(spec, np.float32)
    text_lengths = np.asarray(text_lengths).astype(np.int64)
    spec_lengths = np.asarray(spec_lengths).astype(np.int64)
    text_mask = np.asarray(text_mask, np.float32)
    stm = np.asarray(short_token_mask, np.float32)

    p = {k: params[k] for k in params}
    conv_w = [np.asarray(w, np.float32) for w in p['conv_w']]
    conv_wa = [np.asarray(w, np.float32) for w in p['conv_w_aux']]
    bng_m = np.stack([np.asarray(g, np.float32) for g in p['bn_g']])
    bnb_m = np.stack([np.asarray(b, np.float32) for b in p['bn_b']])
    bng_a = np.stack([np.asarray(g, np.float32) for g in p['bn_g_aux']])
    bnb_a = np.stack([np.asarray(b, np.float32) for b in p['bn_b_aux']])
    emb = np.asarray(p['emb'], np.float32)
    q_w = np.asarray(p['q_w'], np.float32)
    q_b = np.asarray(p['q_b'], np.float32)
    ctc_w = np.asarray(p['ctc_w'], np.float32)
    ctc_b = np.asarray(p['ctc_b'], np.float32)
    sim_w = np.asarray(p['sim_w'], np.float32)
    sim_b = np.asarray(p['sim_b'], np.float32)

    smask = (np.arange(T)[None] < spec_lengths[:, None]).astype(np.float32)
    spec_t = spec.transpose(0, 2, 1) * smask[:, None, :]
    spec_pm = np.zeros((N, MEL, TP), np.float32)
    spec_pm[:, :, 1:1 + T] = spec_t
    mask_b = np.zeros((N, 128, TP), np.float32)
    mask_b[:, :, 1:1 + T] = smask[:, None, :]

    key = emb[text] * text_mask[:, :, None]
    keyn = key / np.maximum(
        np.linalg.norm(key, axis=2, keepdims=True).astype(np.float32), 1e-8)
    keynT = np.ascontiguousarray(keyn.transpose(0, 2, 1), np.float32)

    w0_m = np.ascontiguousarray(
        np.stack([conv_w[0][:, :, k].T for k in range(3)]), np.float32)
    w_m = np.ascontiguousarray(
        np.stack([np.stack([conv_w[l][:, :, k].T for k in range(3)])
                  for l in range(1, 6)]), np.float32)
    w0_a = np.ascontiguousarray(
        np.stack([conv_wa[0][:, :, k].T for k in range(3)]), np.float32)
    w_a = np.ascontiguousarray(
        np.stack([np.stack([conv_wa[l][:, :, k].T for k in range(3)])
                  for l in range(1, 6)]), np.float32)

    shared = dict(
        w0_m=w0_m, w_m=w_m, w0_a=w0_a, w_a=w_a,
        bng_m=np.ascontiguousarray(bng_m[:, :, None], np.float32),
        bnb_m=np.ascontiguousarray(bnb_m[:, :, None], np.float32),
        bng_a=np.ascontiguousarray(bng_a[:, :, None], np.float32),
        bnb_a=np.ascontiguousarray(bnb_a[:, :, None], np.float32),
        qwT=np.ascontiguousarray(q_w.T, np.float32),
        qb=np.ascontiguousarray(q_b[:, None], np.float32),
        ctcwT=np.ascontiguousarray(ctc_w.T, np.float32),
    )
    in_maps = []
    for i in range(NCORES):
        sl = slice(i * NL, (i + 1) * NL)
        m = dict(shared)
        m['spec_pm'] = np.ascontiguousarray(spec_pm[sl])
        m['mask_b'] = np.ascontiguousarray(mask_b[sl])
        m['keynT'] = np.ascontiguousarray(keynT[sl])
        in_maps.append(m)

    host = dict(text=text, spec=spec, text_lengths=text_lengths,
                spec_lengths=spec_lengths, text_mask=text_mask, stm=stm,
                ctc_b=ctc_b, sim_w=sim_w, sim_b=sim_b)
    return in_maps, host


def _log_sigmoid(x):
    return -np.logaddexp(np.float32(0.0), -x)


def _maximum_path(value, t_x, t_y):
    """numpy port of the Glow-TTS monotonic alignment search (f32)."""
    Nb, Tx, Ty = value.shape
    xs = np.arange(Tx)[None]
    dp = np.full((Nb, Tx), NEG, np.float32)
    dp_table = np.empty((Nb, Tx, Ty), np.float32)
    tx = t_x[:, None]
    ty = t_y[:, None]
    for y in range(Ty):
        v = value[:, :, y]
        shifted = np.concatenate(
            [np.full((Nb, 1), NEG, np.float32), dp[:, :-1]], axis=1)
        best = np.maximum(dp, shifted)
        if y == 0:
            base = np.where(xs == 0, np.float32(0.0), np.float32(NEG))
        else:
            base = best
        dp_new = v + base
        valid = (xs <= y) & (xs >= tx + y - ty) & (xs < tx) & (y < ty)
        dp = np.where(valid, dp_new, np.float32(NEG)).astype(np.float32)
        dp_table[:, :, y] = dp
    bi = np.arange(Nb)
    idx = (t_x - 1).astype(np.int64)
    path = np.zeros((Nb, Tx, Ty), np.float32)
    for y in range(Ty - 1, -1, -1):
        active = y < t_y
        col_prev = dp_table[:, :, max(y - 1, 0)]
        v_cur = col_prev[bi, idx]
        v_prev = col_prev[bi, np.maximum(idx - 1, 0)]
        path[bi, idx, y] = active.astype(np.float32)
        move = active & (idx > 0) & (y > 0) & ((idx == y) | (v_cur < v_prev))
        idx = idx - move.astype(np.int64)
    return path


def _ctc_loss_mean(log_probs, targets, in_lens, tgt_lens):
    """numpy port of the reference CTC loss (f32)."""
    Tt, Nb, Cc = log_probs.shape
    S = targets.shape[1]
    L = 2 * S + 1
    z = np.zeros((Nb, L), targets.dtype)
    z[:, 1::2] = targets
    sidx = np.arange(L)[None]
    z_m2 = np.pad(z, ((0, 0), (2, 0)))[:, :L]
    skip = (sidx % 2 == 1) & (sidx >= 2) & (z != z_m2)

    e0 = np.take_along_axis(log_probs[0], z, axis=1)
    alpha = np.full((Nb, L), NEG, np.float32)
    alpha[:, 0] = e0[:, 0]
    alpha[:, 1] = e0[:, 1]
    negpad = np.full((Nb, 1), NEG, np.float32)
    negpad2 = np.full((Nb, 2), NEG, np.float32)
    for t in range(1, Tt):
        lp = log_probs[t]
        a2 = np.concatenate([negpad, alpha[:, :-1]], axis=1)
        a3 = np.where(skip, np.concatenate([negpad2, alpha[:, :-2]], axis=1),
                      np.float32(NEG))
        new = np.take_along_axis(lp, z, axis=1) + np.logaddexp(
            np.logaddexp(alpha, a2), a3)
        alpha = np.where((t < in_lens)[:, None], new, alpha).astype(np.float32)
    bi = np.arange(Nb)
    ll = np.logaddexp(alpha[bi, 2 * tgt_lens], alpha[bi, 2 * tgt_lens - 1])
    return np.mean(-ll / tgt_lens.astype(ll.dtype))


def _finish(D, qn2, ctcT, host):
    """Everything after the device part. D [N,TX,T]; qn2 [N,T]; ctcT [N,V,T]."""
    text = host['text']
    spec = host['spec']
    text_lengths = host['text_lengths']
    spec_lengths = host['spec_lengths']
    stm = host['stm']

    qnorm = np.sqrt(np.maximum(qn2, 0.0)).astype(np.float32)
    cos = D / np.maximum(qnorm, 1e-8)[:, None, :]
    cos = (1.0 - stm) * cos - stm
    similarity = (10.0 * np.exp(host['sim_w']) * cos + host['sim_b']).astype(
        np.float32)

    ctc_logit = ctcT.transpose(0, 2, 1) + host['ctc_b']  # [N, T, V]
    # softmax over V
    mx = ctc_logit.max(axis=2, keepdims=True)
    ex = np.exp(ctc_logit - mx)
    ctc_query = (ex / ex.sum(axis=2, keepdims=True)).astype(np.float32)
    sim_ctc = np.take_along_axis(
        ctc_query.transpose(0, 2, 1), text[:, :, None], axis=1).astype(np.float32)

    tm_b = np.arange(TX)[None] < text_lengths[:, None]
    sm_b = np.arange(T)[None] < spec_lengths[:, None]
    att_mask = (tm_b[:, :, None] & sm_b[:, None, :]).astype(np.float32)

    logsig = _log_sigmoid(similarity).astype(np.float32)
    lsmx = logsig * att_mask
    lsmx_att = lsmx - (lsmx == 0).astype(np.float32) * lsmx.min()
    match_mask = _maximum_path(lsmx_att, text_lengths, spec_lengths)
    attention = match_mask

    lsmx2 = (_log_sigmoid(sim_ctc) * att_mask).astype(np.float32)
    lsmx_aux = lsmx2 - (lsmx2 == 0).astype(np.float32) * lsmx2.min()
    att_aux = _maximum_path(lsmx_aux, text_lengths, spec_lengths)
    aa = np.pad(att_aux, ((0, 0), (1, 1), (0, 0)))
    att_aux = ((aa[:, :-2] + aa[:, 1:-1] + aa[:, 2:]) * att_mask).astype(
        np.float32)

    neg_logsig = _log_sigmoid(-similarity).astype(np.float32)
    denom = att_mask.sum(axis=(1, 2))
    inter = -(match_mask * logsig + (1 - match_mask) * att_mask * neg_logsig)
    nll = np.float32(np.mean(inter.sum(axis=(1, 2)) / denom))
    att_loss = nll
    aux_l = -(att_aux * logsig + (1 - att_aux) * att_mask * neg_logsig)
    att_loss = att_loss + np.float32(
        np.mean(aux_l.sum(axis=(1, 2)) / denom * 0.5))

    # CTC
    lmx = ctc_logit.max(axis=2, keepdims=True)
    lse = lmx + np.log(np.exp(ctc_logit - lmx).sum(axis=2, keepdims=True))
    ctc_in = np.ascontiguousarray(
        (ctc_logit - lse).transpose(1, 0, 2), np.float32)  # [T, N, V]
    att_loss = att_loss + np.float32(
        _ctc_loss_mean(ctc_in, text, spec_lengths, text_lengths))

    # silence promotion
    tm_i = tm_b.astype(np.int32)
    sm_sil = tm_i.copy()
    sm_sil[:, :-1] += -tm_i[:, 1:]
    sm_sil[:, 0] = 1
    silence = sm_sil[:, :, None].astype(np.float32)
    energy = np.mean(np.exp(spec[:, :, 20:]), axis=2)[:, None, :].astype(
        np.float32)
    se_max = np.max(np.sum(energy * silence * attention, axis=1), axis=1)
    ns_min = np.sum(energy * (1 - silence) * attention, axis=1)
    ns_min = np.min((ns_min == 0).astype(np.float32) * 100 + ns_min, axis=1)
    db = ((se_max + ns_min) / 2)[:, None, None]
    promo = (energy <= db).astype(np.float32) * silence
    sp = -0.01 * np.sum(promo * att_mask * logsig, axis=(1, 2)) / np.maximum(
        np.sum(promo * att_mask, axis=(1, 2)), 1.0)
    att_loss = att_loss + np.float32(np.mean(sp))

    return (attention.astype(np.float32), np.float32(att_loss),
            att_mask, np.float32(nll))


# ---------------------------------------------------------------------------
# numpy twin of the device math (for validation without hardware)
# ---------------------------------------------------------------------------

def _device_twin(in_maps):
    """Replicates the device computation in numpy at full-batch level."""
    spec_pm = np.concatenate([m['spec_pm'] for m in in_maps])   # [N, MEL, TP]
    mask_b = np.concatenate([m['mask_b'] for m in in_maps])
    keynT = np.concatenate([m['keynT'] for m in in_maps])
    sh = in_maps[0]

    def stack(w0, w, g, b):
        px = spec_pm.copy()  # [N, K, TP]
        res = None
        for l in range(6):
            wk = [w0[k] for k in range(3)] if l == 0 else \
                 [w[l - 1, k] for k in range(3)]
            conv = np.zeros((N, C, T), np.float32)
            for k in range(3):
                # out[:, oc, t] += wk[k].T @ px[:, ic, t+k]
                conv += np.matmul(wk[k].T[None], px[:, :, k:k + T])
            s1 = conv.sum(axis=(0, 2))
            s2 = (conv * conv).sum(axis=(0, 2))
            mean = s1 / CNT
            var = s2 / CNT - mean * mean
            scl = g[l, :, 0] / np.sqrt(var + EPS)
            bia = b[l, :, 0] - mean * scl
            y = conv * scl[None, :, None] + bia[None, :, None]
            if l % 2 == 1:
                res = y if res is None else (y + res)
                y = res
            if l < 5:
                nxt = np.zeros((N, C, TP), np.float32)
                nxt[:, :, 1:1 + T] = np.maximum(y, 0.0) * mask_b[:, 0:1, 1:1 + T]
                px = nxt
            else:
                return y
        return None

    y5m = stack(sh['w0_m'], sh['w_m'], sh['bng_m'], sh['bnb_m'])
    y5a = stack(sh['w0_a'], sh['w_a'], sh['bng_a'], sh['bnb_a'])

    qT = np.matmul(sh['qwT'].T[None], y5m) + sh['qb'][None, :, :]
    qn2 = (qT * qT).sum(axis=1)
    D = np.matmul(keynT.transpose(0, 2, 1), qT)
    ctcT = np.matmul(sh['ctcwT'].T[None], y5a)
    return D, qn2, ctcT


# ---------------------------------------------------------------------------
# entry point
# ---------------------------------------------------------------------------

def kernel(**inputs):
    global _NC
    in_maps, host = _prep(**inputs)

    if os.environ.get("KERNEL_TWIN"):
        D, qn2, ctcT = _device_twin(in_maps)
        return _finish(D, qn2, ctcT, host)

    from concourse.bass_utils import run_bass_kernel_spmd
    if _NC is None:
        _NC = _build_nc()
    res = run_bass_kernel_spmd(_NC, in_maps, core_ids=list(range(NCORES)))
    outs = res.results
    D = np.concatenate([np.asarray(o['d_out']) for o in outs])
    qn2 = np.concatenate([np.asarray(o['qn2_out']) for o in outs])
    ctcT = np.concatenate([np.asarray(o['ctc_out']) for o in outs])
    return _finish(D, qn2, ctcT, host)
